# revision 17
# baseline (speedup 1.0000x reference)
"""Trainium2 Bass kernel for nn_BiT_Phoneme (dense transformer), v3.

Data-parallel: 16 batch elems / 8 cores = 2 per core. Feature-major
activations ([dim on partitions, tokens on free]); residual x in bf16.
v3 keeps the PE warm through attention (HAM was throttling ~75% of the
attention region in v2):
  - attention heads run as a 3-stage software pipeline per half:
    scores(h) / attnV(h-1) / den-tail(h-2), with one dense filler unit
    (qkv c-tile, v group, or out-proj dtile) per head slot so the PE
    never idles long enough to re-throttle.
  - den tail: reciprocal on the [1,512] den row, then K=1 broadcast
    matmul; of-mul reads the broadcast straight from PSUM (drops the
    dsb copy + [64,512] reciprocal of v2).
  - oden evac moved to ACT identity (DVE was the attention bottleneck).
  - LN stats use DVE add-trees + 2 matmuls instead of 16 stat matmuls.
  - out-proj weights double-pumped: b0 as att1 fillers, b1 after.
"""

import numpy as np

import concourse.bass as bass
import concourse.mybir as mybir
import concourse.tile as tile
from concourse import bacc
from concourse.bass_utils import run_bass_kernel_spmd

B, T, F = 16, 2048, 256
PH = 4
PATCH = 1024
DIM = 1024
DEPTH = 6
HEADS, DHEAD = 16, 64
INNER = 1024
MLP = 4096
NCLS = 41
MAXREL = 200
KSIZE, SIGMA = 20, 2.0
EPS = 1e-5
SEQ = T // PH              # 512
NCORES = 8
BPC = B // NCORES          # 2
TOK = BPC * SEQ            # 1024
P = 128

DT_R = mybir.dt.float32r
DT_F = mybir.dt.float32
DT_H = mybir.dt.bfloat16
FX = mybir.ActivationFunctionType
OP = mybir.AluOpType

DTILES = DIM // P          # 8
KTILES = DIM // P          # 8
MTILES = MLP // P          # 32
SEQT = SEQ // P            # 4


def build_nc(flags):
    nc = bacc.Bacc(None, target_bir_lowering=False)

    par = {}
    def dp(name, shape, dtype, is_out=False):
        par[name] = nc.declare_dram_parameter(name, list(shape), dtype, isOutput=is_out)
        return par[name]

    dp("xin", (BPC, T, F), DT_R)
    dp("band", (T // P, 3, P, P), DT_R)
    dp("etab", (DEPTH, SEQT, P, SEQ), DT_H)
    dp("wpe", (PATCH, DIM), DT_R)
    dp("wqk_t", (DEPTH, 16, P, KTILES, P), DT_H)
    dp("wv_t", (DEPTH, 2, KTILES, P, 512), DT_H)
    dp("wot", (DEPTH, DTILES, P, KTILES, P), DT_H)
    dp("w1t", (DEPTH, MTILES, P, KTILES, P), DT_H)
    dp("w2t", (DEPTH, DTILES, 2, P, 16, P), DT_H)
    dp("wproj", (DIM, NCLS), DT_H)
    for nm, shp in [("lnp1g", (PATCH,)), ("lnp1b", (PATCH,)), ("bpe", (DIM,)),
                    ("lnp2g", (DIM,)), ("lnp2b", (DIM,)),
                    ("lnag", (DEPTH, DIM)), ("lnab", (DEPTH, DIM)),
                    ("lnfg", (DEPTH, DIM)), ("lnfb", (DEPTH, DIM)),
                    ("bov", (DEPTH, DIM)), ("b1v", (DEPTH, MLP)),
                    ("b2v", (DEPTH, DIM)), ("lnog", (DIM,)), ("lnob", (DIM,)),
                    ("bprojv", (NCLS,))]:
        dp(nm, shp, DT_F)
    dp("out", (NCLS, TOK), DT_F, is_out=True)

    with tile.TileContext(nc) as tc:
        _emit(nc, tc, par, flags)
    nc.compile()
    return nc


def _emit(nc, tc, par, flags):
    import contextlib
    ctx = contextlib.ExitStack()
    with ctx:
        const = ctx.enter_context(tc.tile_pool(name="const", bufs=1))
        xpool = ctx.enter_context(tc.tile_pool(name="xpool", bufs=1))
        wsm = ctx.enter_context(tc.tile_pool(name="wsm", bufs=3))
        scr = ctx.enter_context(tc.tile_pool(name="scr", bufs=4))
        scrh = ctx.enter_context(tc.tile_pool(name="scrh", bufs=2))
        stp = ctx.enter_context(tc.tile_pool(name="stp", bufs=2))
        plh = ctx.enter_context(tc.tile_pool(name="plh", bufs=4))
        rowp = ctx.enter_context(tc.tile_pool(name="rowp", bufs=2))
        odnp = ctx.enter_context(tc.tile_pool(name="odnp", bufs=2))
        rdnp = ctx.enter_context(tc.tile_pool(name="rdnp", bufs=2))
        pm = ctx.enter_context(tc.tile_pool(name="pm", bufs=4, space="PSUM"))
        pot = ctx.enter_context(tc.tile_pool(name="pot", bufs=2, space="PSUM"))
        pst = ctx.enter_context(tc.tile_pool(name="pst", bufs=1, space="PSUM"))

        ones_r = const.tile([P, 1], DT_R, name="ones_r")
        nc.vector.memset(ones_r.bitcast(mybir.dt.uint32), 0x3F800000)
        ones_h = const.tile([P, 1], DT_H, name="ones_h")
        nc.vector.memset(ones_h.bitcast(mybir.dt.uint16), 0x3F80)
        ones_row = const.tile([1, P], DT_R, name="ones_row")
        nc.vector.memset(ones_row.bitcast(mybir.dt.uint32), 0x3F800000)
        ones_bc = const.tile([P, 64], DT_H, name="ones_bc")
        nc.vector.memset(ones_bc.bitcast(mybir.dt.uint16), 0x3F80)
        epst = const.tile([1, 1], DT_F, name="epst")
        nc.vector.memset(epst, EPS)

        def load_vec(nm, width):
            d = par[nm]
            if len(d.shape) == 1:
                tl = const.tile([P, width // P], DT_F, name=nm + "_t")
                nc.sync.dma_start(out=tl, in_=d.rearrange("(o p) -> p o", p=P))
            else:
                L = d.shape[0]
                tl = const.tile([P, L, width // P], DT_F, name=nm + "_t")
                nc.sync.dma_start(out=tl, in_=d.rearrange("l (o p) -> p l o", p=P))
            return tl

        lnp1g_t = load_vec("lnp1g", PATCH)
        lnp1b_t = load_vec("lnp1b", PATCH)
        bpe_t = load_vec("bpe", DIM)
        lnp2g_t = load_vec("lnp2g", DIM)
        lnp2b_t = load_vec("lnp2b", DIM)
        lnag_t = load_vec("lnag", DIM)
        lnab_t = load_vec("lnab", DIM)
        lnfg_t = load_vec("lnfg", DIM)
        lnfb_t = load_vec("lnfb", DIM)
        bov_t = load_vec("bov", DIM)
        b1v_t = load_vec("b1v", MLP)
        b2v_t = load_vec("b2v", DIM)
        lnog_t = load_vec("lnog", DIM)
        lnob_t = load_vec("lnob", DIM)
        bproj_t = const.tile([NCLS, 1], DT_F, name="bproj_t")
        nc.sync.dma_start(out=bproj_t,
                          in_=par["bprojv"].rearrange("(p o) -> p o", o=1))

        x = xpool.tile([P, DTILES, TOK], DT_H, name="x")

        # ---------- legacy full-width layernorm (embedding only) ----------
        _lrow_box = []

        def layer_norm_legacy(views, dst_fn, ntiles, D, width, g_fn, b_fn,
                              src_r=True):
            lrow = _lrow_box[0]
            nh = width // 512
            onev = ones_r if src_r else ones_h
            sqdt = DT_R if src_r else DT_H
            st = lrow.tile([1, 2, TOK], DT_R, name="lst")
            mu, rstd = st[:, 0, :width], st[:, 1, :width]
            vt32 = lrow.tile([1, TOK], DT_F, name="lvt")
            vtmp = vt32[:, :width]
            for th in range(nh):
                sl = bass.ts(th, 512)
                ps0 = pst.tile([1, 512], DT_F, name="ps0")
                ps1 = pst.tile([1, 512], DT_F, name="ps1")
                for d in range(ntiles):
                    v = views(d)[:, sl]
                    sq = scr.tile([P, 512], sqdt, name="scr")
                    nc.vector.tensor_mul(sq, v, v)
                    nc.tensor.matmul(ps0, onev, v,
                                     start=(d == 0), stop=(d == ntiles - 1))
                    nc.tensor.matmul(ps1, onev, sq,
                                     start=(d == 0), stop=(d == ntiles - 1))
                nc.vector.tensor_scalar(mu[:, sl], ps0, 1.0 / D, None, OP.mult)
                nc.vector.tensor_scalar(vtmp[:, sl], ps1, 1.0 / D, None,
                                        OP.mult)
                nc.vector.tensor_mul(rstd[:, sl], mu[:, sl], mu[:, sl])
                nc.vector.tensor_sub(vtmp[:, sl], vtmp[:, sl], rstd[:, sl])
            nc.scalar.activation(vtmp, vtmp, FX.Sqrt, bias=epst, scale=1.0)
            lrv = lrow.tile([1, TOK], DT_F, name="lrv")
            nc.vector.reciprocal_approx_fast(out=lrv[:, :width], in_=vtmp)
            nc.vector.tensor_copy(rstd, lrv[:, :width])
            for th in range(nh):
                sl = bass.ts(th, 512)
                sbpl = []
                for rowsl in (mu[:, sl], rstd[:, sl]):
                    pp = pm.tile([P, 512], DT_F, name="pmt")
                    nc.tensor.matmul(pp, ones_row, rowsl,
                                     start=True, stop=True)
                    psb = plh.tile([P, 512], DT_H, name="plh")
                    nc.scalar.activation(psb, pp, FX.Identity)
                    sbpl.append(psb)
                mps, rps = sbpl
                for d in range(ntiles):
                    tmp = scr.tile([P, 512], DT_F, name="scr")
                    nc.vector.tensor_sub(tmp, views(d)[:, sl], mps)
                    nc.vector.tensor_mul(tmp, tmp, rps)
                    nc.vector.tensor_scalar(
                        dst_fn(d)[:, sl], tmp, g_fn(d), b_fn(d),
                        OP.mult, OP.add)

        # =================== embedding ===================
        xin, band = par["xin"], par["band"]
        with (
            tc.tile_pool(name="sfp", bufs=1) as sfp,
            tc.tile_pool(name="pnp", bufs=1) as pnp,
            tc.tile_pool(name="x0p", bufs=1) as x0p,
            tc.tile_pool(name="xap", bufs=16) as xap,
            tc.tile_pool(name="lrow", bufs=1) as lrow_pool,
        ):
            _lrow_box.append(lrow_pool)
            x0 = x0p.tile([P, DTILES, TOK], DT_R, name="x0")
            sf4 = sfp.tile([P, BPC, 2, T], DT_R, name="sf4")
            bfh = [(b, fh) for b in range(BPC) for fh in range(2)]
            xa_cache = {}

            def get_xa(b, kt, fh):
                key = (b, kt, fh)
                if key not in xa_cache:
                    t = xap.tile([P, P], DT_R, name="xa")
                    nc.sync.dma_start(
                        out=t, in_=xin[b, bass.ts(kt, P), bass.ts(fh, P)])
                    xa_cache[key] = t
                return xa_cache[key]

            # prefetch the first few xa tiles before anything else queues
            for kt in range(2):
                for (b, fh) in bfh:
                    get_xa(b, kt, fh)

            for g4 in range(T // 512):
                pgs = {}
                for i, (b, fh) in enumerate(bfh):
                    pool = pm if i < 2 else pot
                    pgs[(b, fh)] = pool.tile([P, 512], DT_F,
                                             name="pmt" if i < 2 else "ot")
                for q in range(4):
                    ct = g4 * 4 + q
                    bt = wsm.tile([P, 3, P], DT_R, name="wsm_t")
                    nc.sync.dma_start(
                        out=bt, in_=band[ct].rearrange("s p q -> p s q"))
                    svals = [s for s in range(3)
                             if 0 <= ct - 1 + s < T // P]
                    for (b, fh) in bfh:
                        for si, s in enumerate(svals):
                            kt = ct - 1 + s
                            xa = get_xa(b, kt, fh)
                            nc.tensor.matmul(
                                pgs[(b, fh)][:, bass.ts(q, P)],
                                xa,
                                bt[:, s, :],
                                start=(q == 0 and si == 0),
                                stop=(q == 3 and si == len(svals) - 1))
                for (b, fh) in bfh:
                    nc.vector.tensor_copy(
                        sf4[:, b, fh, bass.ts(g4, 512)], pgs[(b, fh)])

            pn = pnp.tile([P, BPC, 8, 512], DT_R, name="pn")
            for b in range(BPC):
                def pview(pt, b=b):
                    i, fh = pt // 2, pt % 2
                    return sf4[:, b, fh, :].rearrange(
                        "p (s four) -> p four s", four=PH)[:, i, :]

                layer_norm_legacy(pview, lambda d, b=b: pn[:, b, d, :],
                                  8, PATCH, 512,
                                  lambda d: lnp1g_t[:, d:d + 1],
                                  lambda d: lnp1b_t[:, d:d + 1])

            for dt in range(DTILES):
                wt = wsm.tile([P, KTILES, P], DT_R, name="wsm_t")
                nc.sync.dma_start(
                    out=wt,
                    in_=par["wpe"].rearrange("(ko p) m -> p ko m", p=P)[
                        :, :, bass.ts(dt, P)])
                for b in range(BPC):
                    pq = pm.tile([P, 512], DT_F, name="pmt")
                    for kt in range(KTILES):
                        nc.tensor.matmul(pq, wt[:, kt, :], pn[:, b, kt, :],
                                         start=(kt == 0), stop=(kt == 7))
                    nc.vector.tensor_scalar(
                        x0[:, dt, bass.ts(b, 512)], pq,
                        bpe_t[:, dt:dt + 1], None, OP.add)

            layer_norm_legacy(lambda d: x0[:, d, :], lambda d: x[:, d, :],
                              DTILES, DIM, TOK,
                              lambda d: lnp2g_t[:, d:d + 1],
                              lambda d: lnp2b_t[:, d:d + 1])

        # main pools (opened after embedding scratch closes)
        hpool = ctx.enter_context(tc.tile_pool(name="hpool", bufs=1))
        h2p = ctx.enter_context(tc.tile_pool(name="h2p", bufs=1))
        qkvp = ctx.enter_context(tc.tile_pool(name="qkvp", bufs=2))
        ofp = ctx.enter_context(tc.tile_pool(name="ofp", bufs=1))
        h1p = ctx.enter_context(tc.tile_pool(name="h1p", bufs=1))
        etrp = ctx.enter_context(tc.tile_pool(name="etrp", bufs=2))
        etp = ctx.enter_context(tc.tile_pool(name="etp", bufs=1))

        h = hpool.tile([P, DTILES, TOK], DT_H, name="h")
        of = ofp.tile([P, DTILES, TOK], DT_H, name="of")

        # ---------- split layernorm helpers (main layers) ----------
        def ln_stats(th):
            """PE: paired stat matmuls for token-half th -> (ps0, ps1)."""
            sl = bass.ts(th, 512)
            ps0 = pst.tile([1, 512], DT_F, name="ps0")
            ps1 = pst.tile([1, 512], DT_F, name="ps1")
            for d in range(DTILES):
                sqx = stp.tile([P, 512], DT_H, name="sq")
                nc.vector.tensor_mul(sqx, x[:, d, sl], x[:, d, sl])
                nc.tensor.matmul(ps0, ones_h, x[:, d, sl],
                                 start=(d == 0), stop=(d == DTILES - 1))
                nc.tensor.matmul(ps1, ones_h, sqx,
                                 start=(d == 0), stop=(d == DTILES - 1))
            return ps0, ps1

        def ln_tail(ps0, ps1):
            rows = rowp.tile([1, 2, 512], DT_R, name="rows")
            a = scr.tile([1, 512], DT_F, name="scr")
            v = scr.tile([1, 512], DT_F, name="scr")
            m2 = scr.tile([1, 512], DT_F, name="scr")
            nc.vector.tensor_scalar(a, ps0, 1.0 / DIM, None, OP.mult)
            nc.vector.tensor_scalar(v, ps1, 1.0 / DIM, None, OP.mult)
            nc.vector.tensor_mul(m2, a, a)
            nc.vector.tensor_sub(v, v, m2)
            nc.scalar.activation(v, v, FX.Sqrt, bias=epst, scale=1.0)
            rv = scr.tile([1, 512], DT_F, name="scr")
            nc.vector.reciprocal_approx_fast(out=rv, in_=v)
            nc.vector.tensor_copy(rows[:, 0, :], rv)
            nc.vector.tensor_mul(rows[:, 1, :], a, rv)
            return rows

        def ln_bcast(rows):
            out = []
            for s in range(2):
                pp = pm.tile([P, 512], DT_F, name="pmt")
                nc.tensor.matmul(pp, ones_row, rows[:, s, :],
                                 start=True, stop=True)
                psb = plh.tile([P, 512], DT_H, name="plh")
                nc.scalar.activation(psb, pp, FX.Identity)
                out.append(psb)
            return tuple(out)

        def ln_norm(th, d, planes, dst, g_ap, b_ap, apply_gb):
            sl = bass.ts(th, 512)
            p1, p2 = planes
            t = scrh.tile([P, 512], DT_H, name="lnt")
            nc.vector.tensor_mul(t, x[:, d, sl], p1)
            if apply_gb:
                t2 = scrh.tile([P, 512], DT_H, name="lnt2")
                nc.vector.tensor_sub(t2, t, p2)
                nc.vector.tensor_scalar(dst, t2, g_ap, b_ap, OP.mult, OP.add)
            else:
                nc.vector.tensor_sub(dst, t, p2)

        # =================== transformer layers ===================
        carry_rows_a1 = None
        for l in range(DEPTH):
            gb_a = flags["gb_a"]
            gb_f = flags["gb_f"]

            et_sb = etp.tile([P, SEQT, SEQ], DT_H, name="et_sb")
            nc.sync.dma_start(out=et_sb,
                              in_=par["etab"][l].rearrange("jt p i -> p jt i"))

            if l == 0:
                with nc.named_scope(f"L{l}_lna"):
                    psa0 = ln_stats(0)
                    rows_a0 = ln_tail(*psa0)
                    psa1 = ln_stats(1)
                    rows_a1 = ln_tail(*psa1)
                    planes_a0 = ln_bcast(rows_a0)
            else:
                rows_a1 = carry_rows_a1

            qf = qkvp.tile([P, DTILES, 512], DT_H, name="qf")
            kf = qkvp.tile([P, DTILES, 512], DT_H, name="kf")
            vt4 = qkvp.tile([P, SEQT, HEADS, 65], DT_H, name="vt")
            nc.vector.memset(vt4[:, :, :, 64:65].bitcast(mybir.dt.uint16),
                             0x3F80)
            qf_b = [qf, None]
            kf_b = [kf, None]
            vt_b = [vt4, None]

            def norm_a(th, d, planes=None, lidx=None):
                li = l if lidx is None else lidx
                pl_ = planes if planes is not None else (
                    planes_a0 if th == 0 else planes_a1)
                ln_norm(th, d, pl_, h[:, d, bass.ts(th, 512)],
                        lnag_t[:, li, d:d + 1], lnab_t[:, li, d:d + 1], gb_a)

            qkv_wt_cache = {}

            def qkv_half(b, c):
                cp, ci = c // 2, c % 2
                if ci == 0:
                    wt = wsm.tile([P, 2, KTILES, P], DT_H, name="wsm_t")
                    nc.sync.dma_start(
                        out=wt,
                        in_=par["wqk_t"][l, 2 * cp:2 * cp + 2].rearrange(
                            "c p k m -> p c k m"))
                    qkv_wt_cache[(b, cp)] = wt
                wt = qkv_wt_cache[(b, cp)]
                tsl = bass.ts(b, 512)
                pq = pm.tile([P, 512], DT_F, name="pmt")
                for kt in range(KTILES):
                    nc.tensor.matmul(pq, wt[:, ci, kt, :], h[:, kt, tsl],
                                     start=(kt == 0), stop=(kt == 7))
                if c < DTILES:
                    nc.vector.tensor_copy(qf_b[b][:, c, :], pq)
                else:
                    nc.vector.tensor_scalar(
                        kf_b[b][:, c - DTILES, :], pq,
                        float(DHEAD) ** -0.5, None, OP.mult)

            def out_half(b, dt):
                dp, di = dt // 2, dt % 2
                if di == 0:
                    wt = wsm.tile([P, 2, KTILES, P], DT_H, name="wsm_t")
                    nc.sync.dma_start(
                        out=wt,
                        in_=par["wot"][l, 2 * dp:2 * dp + 2].rearrange(
                            "c p k m -> p c k m"))
                    out_wt_cache[(b, dp)] = wt
                wt = out_wt_cache[(b, dp)]
                tsl = bass.ts(b, 512)
                pq = pm.tile([P, 512], DT_F, name="pmt")
                for kt in range(KTILES):
                    nc.tensor.matmul(
                        pq, wt[:, di, kt, :], of[:, kt, tsl],
                        start=(kt == 0), stop=(kt == 7))
                if flags["bo_nz"]:
                    nc.scalar.activation(pq, pq, FX.Identity,
                                         bias=bov_t[:, l, dt:dt + 1])
                nc.vector.tensor_add(x[:, dt, tsl], pq, x[:, dt, tsl])

            def qkv_pair(b, cp):
                wt = wsm.tile([P, 2, KTILES, P], DT_H, name="wsm_t")
                nc.sync.dma_start(
                    out=wt,
                    in_=par["wqk_t"][l, 2 * cp:2 * cp + 2].rearrange(
                        "c p k m -> p c k m"))
                tsl = bass.ts(b, 512)
                for ci in range(2):
                    c = 2 * cp + ci
                    pq = pm.tile([P, 512], DT_F, name="pmt")
                    for kt in range(KTILES):
                        nc.tensor.matmul(pq, wt[:, ci, kt, :], h[:, kt, tsl],
                                         start=(kt == 0), stop=(kt == 7))
                    if c < DTILES:
                        nc.scalar.activation(qf_b[b][:, c, :], pq, FX.Identity)
                    else:
                        nc.scalar.activation(
                            kf_b[b][:, c - DTILES, :], pq, FX.Identity,
                            scale=float(DHEAD) ** -0.5)

            def v_group(b, nh, tth):
                tts = (2 * tth, 2 * tth + 1)
                pvs = [pm.tile([P, 512], DT_F, name="pmt") for _ in tts]
                for kp in range(KTILES // 2):
                    wv = wsm.tile([P, 2, 512], DT_H, name="wsm_t")
                    nc.sync.dma_start(
                        out=wv,
                        in_=par["wv_t"][l, nh, 2 * kp:2 * kp + 2].rearrange(
                            "k p n -> p k n"))
                    for ki in range(2):
                        kt = 2 * kp + ki
                        for ti, tt in enumerate(tts):
                            nc.tensor.matmul(
                                pvs[ti],
                                h[:, kt, b * 512 + tt * P:
                                  b * 512 + (tt + 1) * P],
                                wv[:, ki, :],
                                start=(kt == 0), stop=(kt == 7))
                for ti, tt in enumerate(tts):
                    nc.scalar.activation(
                        vt_b[b][:, tt, nh * 8:(nh + 1) * 8, 0:64],
                        pvs[ti].rearrange("p (hd d) -> p hd d", d=64),
                        FX.Identity)

            # ---- pipelined attention stages ----
            def attn_scores(b, hd):
                po = (hd % 2) * 64
                dt = hd // 2
                etr = etrp.tile([P, SEQT, 512], DT_H, name="etr")
                for jt in range(SEQT):
                    i0 = jt * P
                    sc = pm.tile([P, 512], DT_F, name="pmt")
                    nc.tensor.matmul(
                        sc[:, i0:],
                        kf_b[b][po:po + 64, dt, bass.ts(jt, P)],
                        qf_b[b][po:po + 64, dt, i0:],
                        start=True, stop=True)
                    ex = scrh.tile([P, 512], DT_H, name="lnt")
                    nc.scalar.activation(ex[:, i0:], sc[:, i0:], FX.Exp)
                    nc.vector.tensor_mul(
                        etr[:, jt, i0:], ex[:, i0:], et_sb[:, jt, i0:])
                return etr

            def attn_av(b, hd, etr):
                ot = pot.tile([P, 512], DT_F, name="ot")
                for jt in range(SEQT):
                    i0 = jt * P
                    nc.tensor.matmul(
                        ot[0:65, i0:],
                        vt_b[b][:, jt, hd, :],
                        etr[:, jt, i0:],
                        start=(jt == 0), stop=(jt == SEQT - 1))
                oden = odnp.tile([65, 512], DT_H, name="oden")
                nc.scalar.activation(oden, ot[0:65, :], FX.Identity)
                return oden

            def attn_tail(b, hd, oden):
                po = (hd % 2) * 64
                dt = hd // 2
                tsl = bass.ts(b, 512)
                pden = pm.tile([64, 512], DT_F, name="pmt")
                nc.tensor.matmul(pden, ones_bc[64:65, :], oden[64:65, :],
                                 start=True, stop=True)
                adrb = rdnp.tile([64, 512], DT_F, name="adrb")
                nc.vector.reciprocal_approx_fast(out=adrb, in_=pden)
                if po == 0:
                    nc.vector.tensor_mul(
                        of[0:64, dt, tsl], oden[0:64, :], adrb)
                else:
                    otmp = scrh.tile([64, 512], DT_H, name="lnt2")
                    nc.vector.tensor_mul(otmp, oden[0:64, :], adrb)
                    nc.sync.dma_start(out=of[64:128, dt, tsl], in_=otmp)

            def attn_pipeline(b, units):
                """Heads 0..15 of half b, one filler unit per head slot."""
                etr_d = {}
                oden_d = {}
                ui = [0]

                def pump():
                    if ui[0] < len(units):
                        units[ui[0]]()
                        ui[0] += 1

                for hd in range(HEADS):
                    if hd >= 1:
                        oden_d[hd - 1] = attn_av(b, hd - 1, etr_d.pop(hd - 1))
                    etr_d[hd] = attn_scores(b, hd)
                    pump()
                    if hd >= 2:
                        attn_tail(b, hd - 2, oden_d.pop(hd - 2))
                oden_d[15] = attn_av(b, 15, etr_d.pop(15))
                attn_tail(b, 14, oden_d.pop(14))
                while ui[0] < len(units):
                    pump()
                attn_tail(b, 15, oden_d.pop(15))

            out_wt_cache = {}

            def out_pair(b, dp):
                wt = wsm.tile([P, 2, KTILES, P], DT_H, name="wsm_t")
                nc.sync.dma_start(
                    out=wt,
                    in_=par["wot"][l, 2 * dp:2 * dp + 2].rearrange(
                        "c p k m -> p c k m"))
                tsl = bass.ts(b, 512)
                for di in range(2):
                    dt = 2 * dp + di
                    pq = pm.tile([P, 512], DT_F, name="pmt")
                    for kt in range(KTILES):
                        nc.tensor.matmul(
                            pq, wt[:, di, kt, :], of[:, kt, tsl],
                            start=(kt == 0), stop=(kt == 7))
                    if flags["bo_nz"]:
                        nc.scalar.activation(pq, pq, FX.Identity,
                                             bias=bov_t[:, l, dt:dt + 1])
                    nc.vector.tensor_add(x[:, dt, tsl], pq, x[:, dt, tsl])

            # ---- phase B: qkv b0 (+ LNa th1 bcast mid-way) + v(b0) ----
            with nc.named_scope(f"L{l}_qkv0"):
                if l == 0:
                    for d in range(DTILES):
                        norm_a(0, d)
                for cp in range(8):
                    qkv_pair(0, cp)
                    if cp == 3:
                        planes_a1 = ln_bcast(rows_a1)
                        for d in range(DTILES):
                            norm_a(1, d)
                for nh in range(2):
                    for tth in range(2):
                        v_group(0, nh, tth)

            # ---- att0: heads b0, fillers = qkv(b1) c-tiles + v(b1) ----
            qf1 = qkvp.tile([P, DTILES, 512], DT_H, name="qf")
            kf1 = qkvp.tile([P, DTILES, 512], DT_H, name="kf")
            vt41 = qkvp.tile([P, SEQT, HEADS, 65], DT_H, name="vt")
            nc.vector.memset(vt41[:, :, :, 64:65].bitcast(mybir.dt.uint16),
                             0x3F80)
            qf_b[1], kf_b[1], vt_b[1] = qf1, kf1, vt41

            units0 = (
                [lambda c=c: qkv_half(1, c)
                 for c in (0, 1, 2, 3, 4, 5, 8, 9, 10, 11, 12, 13)]
                + [lambda nh=nh, tth=tth: v_group(1, nh, tth)
                   for nh in range(2) for tth in range(2)]
            )
            with nc.named_scope(f"L{l}_att0"):
                attn_pipeline(0, units0)

            # ---- att1: heads b1, fillers = deferred qkv(b1) + out(b0) ----
            units1 = (
                [lambda c=c: qkv_half(1, c) for c in (6, 14, 7, 15)]
                + [lambda dt=dt: out_half(0, dt) for dt in range(DTILES)]
            )
            with nc.named_scope(f"L{l}_att1"):
                attn_pipeline(1, units1)

            # ---- out tail: LNf stats + out b1 ----
            with nc.named_scope(f"L{l}_out1"):
                psf0 = ln_stats(0)
                rows_f0 = ln_tail(*psf0)
                for dp in range(2):
                    out_pair(1, dp)
                planes_f0 = ln_bcast(rows_f0)
                for dp in range(2, 4):
                    out_pair(1, dp)
                psf1 = ln_stats(1)
                rows_f1 = ln_tail(*psf1)

            h2 = h2p.tile([P, DTILES, 512], DT_H, name="h2")
            h1r = h1p.tile([P, MTILES, 512], DT_H, name="h1r")

            def norm_f(th, d, planes):
                ln_norm(th, d, planes, h2[:, d, :],
                        lnfg_t[:, l, d:d + 1], lnfb_t[:, l, d:d + 1], gb_f)

            def w1_pass(th):
                for mp in range(MTILES // 2):
                    wt = wsm.tile([P, 2, KTILES, P], DT_H, name="wsm_t")
                    nc.sync.dma_start(
                        out=wt,
                        in_=par["w1t"][l, 2 * mp:2 * mp + 2].rearrange(
                            "c p k m -> p c k m"))
                    for mi in range(2):
                        mt = 2 * mp + mi
                        pq = pm.tile([P, 512], DT_F, name="pmt")
                        for kt in range(KTILES):
                            nc.tensor.matmul(
                                pq, wt[:, mi, kt, :], h2[:, kt, :],
                                start=(kt == 0), stop=(kt == 7))
                        nc.scalar.activation(
                            h1r[:, mt, :], pq, FX.Gelu,
                            bias=b1v_t[:, l, mt:mt + 1], scale=1.0)

            def w2_pass(th):
                tsl = bass.ts(th, 512)
                for dt in range(DTILES):
                    pq = pm.tile([P, 512], DT_F, name="pmt")
                    for kh in range(2):
                        wt = wsm.tile([P, 16, P], DT_H, name="wsm_t")
                        nc.sync.dma_start(out=wt, in_=par["w2t"][l, dt, kh])
                        for k2 in range(16):
                            kt = kh * 16 + k2
                            nc.tensor.matmul(
                                pq, wt[:, k2, :], h1r[:, kt, :],
                                start=(kt == 0), stop=(kt == 31))
                    if flags["b2_nz"]:
                        nc.scalar.activation(pq, pq, FX.Identity,
                                             bias=b2v_t[:, l, dt:dt + 1])
                    nc.vector.tensor_add(x[:, dt, tsl], pq, x[:, dt, tsl])

            with nc.named_scope(f"L{l}_ffn"):
                for d in range(DTILES):
                    norm_f(0, d, planes_f0)
                w1_pass(0)
                pf1 = ln_bcast(rows_f1)
                for d in range(DTILES):
                    norm_f(1, d, pf1)
                w2_pass(0)
                if l < DEPTH - 1:
                    psn0 = ln_stats(0)
                    rows_n0 = ln_tail(*psn0)
                w1_pass(1)
                if l < DEPTH - 1:
                    pn0 = ln_bcast(rows_n0)
                    for d in range(DTILES):
                        norm_a(0, d, planes=pn0, lidx=l + 1)
                w2_pass(1)
                if l < DEPTH - 1:
                    psn1 = ln_stats(1)
                    carry_rows_a1 = ln_tail(*psn1)

        # =================== head ===================
        for th in range(2):
            ps = ln_stats(th)
            rows_o = ln_tail(*ps)
            pl_o = ln_bcast(rows_o)
            for d in range(DTILES):
                ln_norm(th, d, pl_o, h[:, d, bass.ts(th, 512)],
                        lnog_t[:, d:d + 1], lnob_t[:, d:d + 1],
                        flags["gb_o"])
        wp3 = par["wproj"].rearrange("(ko p) m -> p ko m", p=P)
        wt = wsm.tile([P, KTILES, NCLS], DT_H, name="wsm_t")
        nc.sync.dma_start(out=wt, in_=wp3)
        out_sb = h2p.tile([NCLS, TOK], DT_F, name="h2")
        for th in range(2):
            pq = pot.tile([P, 512], DT_F, name="ot")
            for kt in range(KTILES):
                nc.tensor.matmul(pq[0:NCLS, :], wt[:, kt, :],
                                 h[:, kt, bass.ts(th, 512)],
                                 start=(kt == 0), stop=(kt == 7))
            nc.scalar.activation(out_sb[:, bass.ts(th, 512)],
                                 pq[0:NCLS, :],
                                 FX.Identity, bias=bproj_t)
        nc.sync.dma_start(out=par["out"][:, :], in_=out_sb)


# ============================================================
# host side
# ============================================================

_NC_CACHE = None
_NC_FLAGS = None


def _bf16(a):
    import ml_dtypes
    return np.ascontiguousarray(a.astype(ml_dtypes.bfloat16))


def _pack_qk(w):      # [D, DIM, 3072] -> [D, 16, P, 8, P]
    v = w[:, :, :2048].reshape(DEPTH, 8, P, 16, P).transpose(0, 3, 2, 1, 4)
    return _bf16(v)


def _pack_v(w):       # -> [D, 2, 8, P, 512]
    v = w[:, :, 2048:].reshape(DEPTH, 8, P, 2, 512).transpose(0, 3, 1, 2, 4)
    return _bf16(v)


def _pack_kxm(w):     # [D, K, M] -> [D, M//P, P, K//P, P]
    D, K, M = w.shape
    v = w.reshape(D, K // P, P, M // P, P).transpose(0, 3, 2, 1, 4)
    return _bf16(v)


def _pack_w2(w):      # [D, 4096, 1024] -> [D, 8, 2, P, 16, P]
    v = w.reshape(DEPTH, 2, 16, P, 8, P).transpose(0, 4, 1, 3, 2, 5)
    return _bf16(v)


def _host_band():
    tt = np.arange(KSIZE, dtype=np.float64)
    kern = np.exp(-0.5 * ((tt - (KSIZE - 1) / 2.0) / SIGMA) ** 2)
    kern = (kern / kern.sum()).astype(np.float32)
    pad_l = (KSIZE - 1) // 2  # 9
    nt = T // P
    bandc = np.zeros((nt, 3, P, P), dtype=np.float32)
    for ct in range(nt):
        for s in range(3):
            kt = ct - 1 + s
            if not (0 <= kt < nt):
                continue
            rows = np.arange(kt * P, (kt + 1) * P)
            cols = np.arange(ct * P, (ct + 1) * P)
            d = rows[:, None] - cols[None, :] + pad_l
            m = (d >= 0) & (d < KSIZE)
            blk = np.zeros((P, P), np.float32)
            blk[m] = kern[d[m]]
            bandc[ct, s] = blk
    return bandc


def _host_etab(rel_tab):
    i = np.arange(SEQ)
    j = i[:, None]
    rel = np.clip(i[None, :] - j, -(MAXREL - 1), MAXREL - 1) + MAXREL - 1
    et = np.zeros((DEPTH, SEQ, SEQ), dtype=np.float32)
    for l in range(DEPTH):
        e = np.exp(rel_tab[l][rel])
        e[j > i[None, :]] = 0.0
        et[l] = e
    return _bf16(et.reshape(DEPTH, SEQT, P, SEQ))


def kernel(**inputs):
    global _NC_CACHE, _NC_FLAGS

    f32 = lambda a: np.ascontiguousarray(np.asarray(a, dtype=np.float32))
    z = lambda a: bool(np.any(np.asarray(a) != 0))
    one = lambda a: bool(np.all(np.asarray(a) == 1.0))
    flags = {
        "gb_a": (not one(inputs["ln_a_g"])) or z(inputs["ln_a_b"]),
        "gb_f": (not one(inputs["ln_f_g"])) or z(inputs["ln_f_b"]),
        "gb_o": (not one(inputs["ln_o_g"])) or z(inputs["ln_o_b"]),
        "bo_nz": z(inputs["bo"]),
        "b2_nz": z(inputs["b2"]),
    }
    if _NC_CACHE is None or _NC_FLAGS != flags:
        _NC_CACHE = build_nc(flags)
        _NC_FLAGS = dict(flags)
    nc = _NC_CACHE

    shared = {
        "band": _host_band(),
        "etab": _host_etab(f32(inputs["rel_tab"])),
        "wpe": f32(inputs["W_pe"]),
        "wqk_t": _pack_qk(f32(inputs["Wqkv"])),
        "wv_t": _pack_v(f32(inputs["Wqkv"])),
        "wot": _pack_kxm(f32(inputs["Wo"])),
        "w1t": _pack_kxm(f32(inputs["W1"])),
        "w2t": _pack_w2(f32(inputs["W2"])),
        "wproj": _bf16(f32(inputs["Wproj"])),
        "lnp1g": f32(inputs["ln_p1_g"]), "lnp1b": f32(inputs["ln_p1_b"]),
        "bpe": f32(inputs["b_pe"]),
        "lnp2g": f32(inputs["ln_p2_g"]), "lnp2b": f32(inputs["ln_p2_b"]),
        "lnag": f32(inputs["ln_a_g"]), "lnab": f32(inputs["ln_a_b"]),
        "lnfg": f32(inputs["ln_f_g"]), "lnfb": f32(inputs["ln_f_b"]),
        "bov": f32(inputs["bo"]), "b1v": f32(inputs["b1"]),
        "b2v": f32(inputs["b2"]),
        "lnog": f32(inputs["ln_o_g"]), "lnob": f32(inputs["ln_o_b"]),
        "bprojv": f32(inputs["bproj"]),
    }
    xfull = f32(inputs["neuralInput"])
    in_maps = []
    for c in range(NCORES):
        m = dict(shared)
        m["xin"] = np.ascontiguousarray(xfull[c * BPC:(c + 1) * BPC])
        in_maps.append(m)

    import os
    trace = bool(os.environ.get("BIT_TRACE"))
    res = run_bass_kernel_spmd(nc, in_maps, list(range(NCORES)), trace=trace)
    if trace:
        globals()["LAST_RESULT"] = res
    outs = []
    for c in range(NCORES):
        o = res.results[c]["out"]              # [NCLS, TOK]
        o = o.reshape(NCLS, BPC, SEQ).transpose(1, 2, 0)
        outs.append(o)
    return np.concatenate(outs, axis=0).astype(np.float32)


# revision 23
# speedup vs baseline: 1.2210x; 1.2210x over previous
"""Trainium2 Bass kernel for nn_BiT_Phoneme (dense transformer), v3.

Data-parallel: 16 batch elems / 8 cores = 2 per core. Feature-major
activations ([dim on partitions, tokens on free]); residual x in bf16.
v3 keeps the PE warm through attention (HAM was throttling ~75% of the
attention region in v2):
  - attention heads run as a 3-stage software pipeline per half:
    scores(h) / attnV(h-1) / den-tail(h-2), with one dense filler unit
    (qkv c-tile, v group, or out-proj dtile) per head slot so the PE
    never idles long enough to re-throttle.
  - den tail: reciprocal on the [1,512] den row, then K=1 broadcast
    matmul; of-mul reads the broadcast straight from PSUM (drops the
    dsb copy + [64,512] reciprocal of v2).
  - oden evac moved to ACT identity (DVE was the attention bottleneck).
  - LN stats use DVE add-trees + 2 matmuls instead of 16 stat matmuls.
  - out-proj weights double-pumped: b0 as att1 fillers, b1 after.
"""

import numpy as np

import concourse.bass as bass
import concourse.mybir as mybir
import concourse.tile as tile
from concourse import bacc
from concourse.bass_utils import run_bass_kernel_spmd

B, T, F = 16, 2048, 256
PH = 4
PATCH = 1024
DIM = 1024
DEPTH = 6
HEADS, DHEAD = 16, 64
INNER = 1024
MLP = 4096
NCLS = 41
MAXREL = 200
KSIZE, SIGMA = 20, 2.0
EPS = 1e-5
SEQ = T // PH              # 512
NCORES = 8
BPC = B // NCORES          # 2
TOK = BPC * SEQ            # 1024
P = 128

DT_R = mybir.dt.float32r
DT_F = mybir.dt.float32
DT_H = mybir.dt.bfloat16
FX = mybir.ActivationFunctionType
OP = mybir.AluOpType

DTILES = DIM // P          # 8
KTILES = DIM // P          # 8
MTILES = MLP // P          # 32
SEQT = SEQ // P            # 4


def build_nc(flags):
    nc = bacc.Bacc(None, target_bir_lowering=False)

    par = {}
    def dp(name, shape, dtype, is_out=False):
        par[name] = nc.declare_dram_parameter(name, list(shape), dtype, isOutput=is_out)
        return par[name]

    dp("xin", (BPC, T, F), DT_R)
    dp("band", (T // P, 3, P, P), DT_R)
    dp("etab", (DEPTH, SEQT, P, SEQ), DT_H)
    dp("wpe", (PATCH, DIM), DT_R)
    dp("wqk_t", (DEPTH, 16, P, KTILES, P), DT_H)
    dp("wv_t", (DEPTH, 2, KTILES, P, 512), DT_H)
    dp("wot", (DEPTH, DTILES, P, KTILES, P), DT_H)
    dp("w1t", (DEPTH, MTILES, P, KTILES, P), DT_H)
    dp("w2t", (DEPTH, DTILES, 2, P, 16, P), DT_H)
    dp("wproj", (DIM, NCLS), DT_H)
    # vectors arrive pre-transposed to [P, (L,) width//P] so the DMA is
    # one contiguous run per partition instead of an element gather
    for nm, width, L in [("lnp1g", PATCH, 0), ("lnp1b", PATCH, 0),
                         ("bpe", DIM, 0), ("lnp2g", DIM, 0),
                         ("lnp2b", DIM, 0),
                         ("lnag", DIM, DEPTH), ("lnab", DIM, DEPTH),
                         ("lnfg", DIM, DEPTH), ("lnfb", DIM, DEPTH),
                         ("bov", DIM, DEPTH), ("b1v", MLP, DEPTH),
                         ("b2v", DIM, DEPTH), ("lnog", DIM, 0),
                         ("lnob", DIM, 0)]:
        shp = (P, L, width // P) if L else (P, width // P)
        dp(nm, shp, DT_F)
    dp("bprojv", (NCLS,), DT_F)
    dp("out", (NCLS, TOK), DT_F, is_out=True)

    with tile.TileContext(nc) as tc:
        _emit(nc, tc, par, flags)
    nc.compile()
    return nc


def _emit(nc, tc, par, flags):
    import contextlib
    ctx = contextlib.ExitStack()
    with ctx:
        const = ctx.enter_context(tc.tile_pool(name="const", bufs=1))
        xpool = ctx.enter_context(tc.tile_pool(name="xpool", bufs=1))
        wsm = ctx.enter_context(tc.tile_pool(name="wsm", bufs=4))
        scr = ctx.enter_context(tc.tile_pool(name="scr", bufs=4))
        scrh = ctx.enter_context(tc.tile_pool(name="scrh", bufs=2))
        stp = ctx.enter_context(tc.tile_pool(name="stp", bufs=2))
        plh = ctx.enter_context(tc.tile_pool(name="plh", bufs=4))
        rowp = ctx.enter_context(tc.tile_pool(name="rowp", bufs=2))
        odnp = ctx.enter_context(tc.tile_pool(name="odnp", bufs=2))
        rdnp = ctx.enter_context(tc.tile_pool(name="rdnp", bufs=2))
        pm = ctx.enter_context(tc.tile_pool(name="pm", bufs=4, space="PSUM"))
        pot = ctx.enter_context(tc.tile_pool(name="pot", bufs=2, space="PSUM"))
        pst = ctx.enter_context(tc.tile_pool(name="pst", bufs=1, space="PSUM"))

        ones_r = const.tile([P, 1], DT_R, name="ones_r")
        nc.vector.memset(ones_r.bitcast(mybir.dt.uint32), 0x3F800000)
        ones_h = const.tile([P, 1], DT_H, name="ones_h")
        nc.vector.memset(ones_h.bitcast(mybir.dt.uint16), 0x3F80)
        ones_row = const.tile([1, P], DT_R, name="ones_row")
        nc.vector.memset(ones_row.bitcast(mybir.dt.uint32), 0x3F800000)
        ones_bc = const.tile([P, 64], DT_H, name="ones_bc")
        nc.vector.memset(ones_bc.bitcast(mybir.dt.uint16), 0x3F80)
        epst = const.tile([1, 1], DT_F, name="epst")
        nc.vector.memset(epst, EPS)

        def load_vec(nm, width):
            d = par[nm]
            tl = const.tile(list(d.shape), DT_F, name=nm + "_t")
            nc.sync.dma_start(out=tl, in_=d[tuple(slice(None) for _ in d.shape)])
            return tl

        lnp1g_t = load_vec("lnp1g", PATCH)
        lnp1b_t = load_vec("lnp1b", PATCH)
        bpe_t = load_vec("bpe", DIM)
        lnp2g_t = load_vec("lnp2g", DIM)
        lnp2b_t = load_vec("lnp2b", DIM)
        lnag_t = load_vec("lnag", DIM)
        lnab_t = load_vec("lnab", DIM)
        lnfg_t = load_vec("lnfg", DIM)
        lnfb_t = load_vec("lnfb", DIM)
        bov_t = load_vec("bov", DIM)
        b1v_t = load_vec("b1v", MLP)
        b2v_t = load_vec("b2v", DIM)
        lnog_t = load_vec("lnog", DIM)
        lnob_t = load_vec("lnob", DIM)
        bproj_t = const.tile([NCLS, 1], DT_F, name="bproj_t")
        nc.sync.dma_start(out=bproj_t,
                          in_=par["bprojv"].rearrange("(p o) -> p o", o=1))

        x = xpool.tile([P, DTILES, TOK], DT_H, name="x")

        # ---------- legacy full-width layernorm (embedding only) ----------
        _lrow_box = []

        def layer_norm_legacy(views, dst_fn, ntiles, D, width, g_fn, b_fn,
                              src_r=True):
            lrow = _lrow_box[0]
            nh = width // 512
            onev = ones_r if src_r else ones_h
            sqdt = DT_R if src_r else DT_H
            st = lrow.tile([1, 2, TOK], DT_R, name="lst")
            mu, rstd = st[:, 0, :width], st[:, 1, :width]
            vt32 = lrow.tile([1, TOK], DT_F, name="lvt")
            vtmp = vt32[:, :width]
            for th in range(nh):
                sl = bass.ts(th, 512)
                ps0 = pst.tile([1, 512], DT_F, name="ps0")
                ps1 = pst.tile([1, 512], DT_F, name="ps1")
                for d in range(ntiles):
                    v = views(d)[:, sl]
                    sq = scr.tile([P, 512], sqdt, name="scr")
                    nc.vector.tensor_mul(sq, v, v)
                    nc.tensor.matmul(ps0, onev, v,
                                     start=(d == 0), stop=(d == ntiles - 1))
                    nc.tensor.matmul(ps1, onev, sq,
                                     start=(d == 0), stop=(d == ntiles - 1))
                nc.vector.tensor_scalar(mu[:, sl], ps0, 1.0 / D, None, OP.mult)
                nc.vector.tensor_scalar(vtmp[:, sl], ps1, 1.0 / D, None,
                                        OP.mult)
                nc.vector.tensor_mul(rstd[:, sl], mu[:, sl], mu[:, sl])
                nc.vector.tensor_sub(vtmp[:, sl], vtmp[:, sl], rstd[:, sl])
            nc.scalar.activation(vtmp, vtmp, FX.Sqrt, bias=epst, scale=1.0)
            lrv = lrow.tile([1, TOK], DT_F, name="lrv")
            nc.vector.reciprocal_approx_fast(out=lrv[:, :width], in_=vtmp)
            nc.vector.tensor_copy(rstd, lrv[:, :width])
            for th in range(nh):
                sl = bass.ts(th, 512)
                sbpl = []
                for rowsl in (mu[:, sl], rstd[:, sl]):
                    pp = pm.tile([P, 512], DT_F, name="pmt")
                    nc.tensor.matmul(pp, ones_row, rowsl,
                                     start=True, stop=True)
                    psb = plh.tile([P, 512], DT_H, name="plh")
                    nc.scalar.activation(psb, pp, FX.Identity)
                    sbpl.append(psb)
                mps, rps = sbpl
                for d in range(ntiles):
                    tmp = scr.tile([P, 512], DT_F, name="scr")
                    nc.vector.tensor_sub(tmp, views(d)[:, sl], mps)
                    nc.vector.tensor_mul(tmp, tmp, rps)
                    nc.vector.tensor_scalar(
                        dst_fn(d)[:, sl], tmp, g_fn(d), b_fn(d),
                        OP.mult, OP.add)

        # =================== embedding ===================
        xin, band = par["xin"], par["band"]
        with (
            tc.tile_pool(name="sfp", bufs=1) as sfp,
            tc.tile_pool(name="pnp", bufs=1) as pnp,
            tc.tile_pool(name="x0p", bufs=1) as x0p,
            tc.tile_pool(name="xap", bufs=16) as xap,
            tc.tile_pool(name="lrow", bufs=1) as lrow_pool,
        ):
            _lrow_box.append(lrow_pool)
            x0 = x0p.tile([P, DTILES, TOK], DT_R, name="x0")
            sf4 = sfp.tile([P, BPC, 2, T], DT_R, name="sf4")
            bfh = [(b, fh) for b in range(BPC) for fh in range(2)]
            xa_cache = {}

            def get_xa(b, kt, fh):
                key = (b, kt, fh)
                if key not in xa_cache:
                    t = xap.tile([P, P], DT_R, name="xa")
                    nc.sync.dma_start(
                        out=t, in_=xin[b, bass.ts(kt, P), bass.ts(fh, P)])
                    xa_cache[key] = t
                return xa_cache[key]

            # prefetch the first few xa tiles before anything else queues
            for kt in range(2):
                for (b, fh) in bfh:
                    get_xa(b, kt, fh)

            for g4 in range(T // 512):
                pgs = {}
                for i, (b, fh) in enumerate(bfh):
                    pool = pm if i < 2 else pot
                    pgs[(b, fh)] = pool.tile([P, 512], DT_F,
                                             name="pmt" if i < 2 else "ot")
                for q in range(4):
                    ct = g4 * 4 + q
                    bt = wsm.tile([P, 3, P], DT_R, name="wsm_t")
                    nc.sync.dma_start(
                        out=bt, in_=band[ct].rearrange("s p q -> p s q"))
                    svals = [s for s in range(3)
                             if 0 <= ct - 1 + s < T // P]
                    for (b, fh) in bfh:
                        for si, s in enumerate(svals):
                            kt = ct - 1 + s
                            xa = get_xa(b, kt, fh)
                            nc.tensor.matmul(
                                pgs[(b, fh)][:, bass.ts(q, P)],
                                xa,
                                bt[:, s, :],
                                start=(q == 0 and si == 0),
                                stop=(q == 3 and si == len(svals) - 1))
                for (b, fh) in bfh:
                    nc.vector.tensor_copy(
                        sf4[:, b, fh, bass.ts(g4, 512)], pgs[(b, fh)])

            pn = pnp.tile([P, BPC, 8, 512], DT_R, name="pn")
            for b in range(BPC):
                def pview(pt, b=b):
                    i, fh = pt // 2, pt % 2
                    return sf4[:, b, fh, :].rearrange(
                        "p (s four) -> p four s", four=PH)[:, i, :]

                layer_norm_legacy(pview, lambda d, b=b: pn[:, b, d, :],
                                  8, PATCH, 512,
                                  lambda d: lnp1g_t[:, d:d + 1],
                                  lambda d: lnp1b_t[:, d:d + 1])

            for dt in range(DTILES):
                wt = wsm.tile([P, KTILES, P], DT_R, name="wsm_t")
                nc.sync.dma_start(
                    out=wt,
                    in_=par["wpe"].rearrange("(ko p) m -> p ko m", p=P)[
                        :, :, bass.ts(dt, P)])
                for b in range(BPC):
                    pq = pm.tile([P, 512], DT_F, name="pmt")
                    for kt in range(KTILES):
                        nc.tensor.matmul(pq, wt[:, kt, :], pn[:, b, kt, :],
                                         start=(kt == 0), stop=(kt == 7))
                    nc.vector.tensor_scalar(
                        x0[:, dt, bass.ts(b, 512)], pq,
                        bpe_t[:, dt:dt + 1], None, OP.add)

            layer_norm_legacy(lambda d: x0[:, d, :], lambda d: x[:, d, :],
                              DTILES, DIM, TOK,
                              lambda d: lnp2g_t[:, d:d + 1],
                              lambda d: lnp2b_t[:, d:d + 1])

        # main pools (opened after embedding scratch closes)
        hpool = ctx.enter_context(tc.tile_pool(name="hpool", bufs=1))
        h2p = ctx.enter_context(tc.tile_pool(name="h2p", bufs=1))
        qkvp = ctx.enter_context(tc.tile_pool(name="qkvp", bufs=2))
        ofp = ctx.enter_context(tc.tile_pool(name="ofp", bufs=1))
        h1p = ctx.enter_context(tc.tile_pool(name="h1p", bufs=1))
        etrp = ctx.enter_context(tc.tile_pool(name="etrp", bufs=2))
        etp = ctx.enter_context(tc.tile_pool(name="etp", bufs=1))

        h = hpool.tile([P, DTILES, TOK], DT_H, name="h")
        of = ofp.tile([P, DTILES, TOK], DT_H, name="of")

        # ---------- split layernorm helpers (main layers) ----------
        def ln_stats(th):
            """DVE add-chains for sum/sumsq + 2 partition-reduce matmuls."""
            sl = bass.ts(th, 512)
            ps0 = pst.tile([1, 512], DT_F, name="ps0")
            ps1 = pst.tile([1, 512], DT_F, name="ps1")

            flip = [0]

            def acc_tile():
                t = stp.tile([P, 512], DT_H,
                             name=("ta" if flip[0] else "tb"))
                flip[0] ^= 1
                return t

            sacc = x[:, 0, sl]
            for d in range(1, DTILES):
                t = acc_tile()
                nc.vector.tensor_add(t, sacc, x[:, d, sl])
                sacc = t
            nc.tensor.matmul(ps0, ones_h, sacc, start=True, stop=True)
            qacc = None
            for d in range(DTILES):
                q = stp.tile([P, 512], DT_H, name="sq")
                nc.vector.tensor_mul(q, x[:, d, sl], x[:, d, sl])
                if qacc is None:
                    qacc = q
                else:
                    t = acc_tile()
                    nc.vector.tensor_add(t, qacc, q)
                    qacc = t
            nc.tensor.matmul(ps1, ones_h, qacc, start=True, stop=True)
            return ps0, ps1

        def ln_tail(ps0, ps1):
            rows = rowp.tile([1, 2, 512], DT_R, name="rows")
            a = scr.tile([1, 512], DT_F, name="scr")
            v = scr.tile([1, 512], DT_F, name="scr")
            m2 = scr.tile([1, 512], DT_F, name="scr")
            nc.vector.tensor_scalar(a, ps0, 1.0 / DIM, None, OP.mult)
            nc.vector.tensor_scalar(v, ps1, 1.0 / DIM, None, OP.mult)
            nc.vector.tensor_mul(m2, a, a)
            nc.vector.tensor_sub(v, v, m2)
            nc.scalar.activation(v, v, FX.Sqrt, bias=epst, scale=1.0)
            rv = scr.tile([1, 512], DT_F, name="scr")
            nc.vector.reciprocal_approx_fast(out=rv, in_=v)
            nc.vector.tensor_copy(rows[:, 0, :], rv)
            nc.vector.tensor_mul(rows[:, 1, :], a, rv)
            return rows

        def ln_bcast(rows):
            out = []
            for s in range(2):
                pp = pm.tile([P, 512], DT_F, name="pmt")
                nc.tensor.matmul(pp, ones_row, rows[:, s, :],
                                 start=True, stop=True)
                psb = plh.tile([P, 512], DT_H, name="plh")
                nc.scalar.activation(psb, pp, FX.Identity)
                out.append(psb)
            return tuple(out)

        def ln_norm(th, d, planes, dst, g_ap, b_ap, apply_gb):
            sl = bass.ts(th, 512)
            p1, p2 = planes
            t = scrh.tile([P, 512], DT_H, name="lnt")
            nc.vector.tensor_mul(t, x[:, d, sl], p1)
            if apply_gb:
                t2 = scrh.tile([P, 512], DT_H, name="lnt2")
                nc.vector.tensor_sub(t2, t, p2)
                nc.vector.tensor_scalar(dst, t2, g_ap, b_ap, OP.mult, OP.add)
            else:
                nc.vector.tensor_sub(dst, t, p2)

        # =================== transformer layers ===================
        carry_rows_a1 = None
        for l in range(DEPTH):
            gb_a = flags["gb_a"]
            gb_f = flags["gb_f"]

            et_sb = etp.tile([P, SEQT, SEQ], DT_H, name="et_sb")
            nc.sync.dma_start(out=et_sb,
                              in_=par["etab"][l].rearrange("jt p i -> p jt i"))

            if l == 0:
                with nc.named_scope(f"L{l}_lna"):
                    psa0 = ln_stats(0)
                    rows_a0 = ln_tail(*psa0)
                    psa1 = ln_stats(1)
                    rows_a1 = ln_tail(*psa1)
                    planes_a0 = ln_bcast(rows_a0)
            else:
                rows_a1 = carry_rows_a1

            qf = qkvp.tile([P, DTILES, 512], DT_H, name="qf")
            kf = qkvp.tile([P, DTILES, 512], DT_H, name="kf")
            vt4 = qkvp.tile([P, SEQT, HEADS, 65], DT_H, name="vt")
            nc.vector.memset(vt4[:, :, :, 64:65].bitcast(mybir.dt.uint16),
                             0x3F80)
            qf_b = [qf, None]
            kf_b = [kf, None]
            vt_b = [vt4, None]

            def norm_a(th, d, planes=None, lidx=None):
                li = l if lidx is None else lidx
                pl_ = planes if planes is not None else (
                    planes_a0 if th == 0 else planes_a1)
                ln_norm(th, d, pl_, h[:, d, bass.ts(th, 512)],
                        lnag_t[:, li, d:d + 1], lnab_t[:, li, d:d + 1], gb_a)

            qkv_wt_cache = {}

            def qkv_half(b, c):
                cp, ci = c // 2, c % 2
                if ci == 0:
                    wt = wsm.tile([P, 2, KTILES, P], DT_H, name="wsm_t")
                    nc.sync.dma_start(
                        out=wt,
                        in_=par["wqk_t"][l, 2 * cp:2 * cp + 2].rearrange(
                            "c p k m -> p c k m"))
                    qkv_wt_cache[(b, cp)] = wt
                wt = qkv_wt_cache[(b, cp)]
                tsl = bass.ts(b, 512)
                pq = pm.tile([P, 512], DT_F, name="pmt")
                for kt in range(KTILES):
                    nc.tensor.matmul(pq, wt[:, ci, kt, :], h[:, kt, tsl],
                                     start=(kt == 0), stop=(kt == 7))
                if c < DTILES:
                    nc.vector.tensor_copy(qf_b[b][:, c, :], pq)
                else:
                    nc.vector.tensor_scalar(
                        kf_b[b][:, c - DTILES, :], pq,
                        float(DHEAD) ** -0.5, None, OP.mult)

            def out_half(b, dt):
                dp, di = dt // 2, dt % 2
                if di == 0:
                    wt = wsm.tile([P, 2, KTILES, P], DT_H, name="wsm_t")
                    nc.sync.dma_start(
                        out=wt,
                        in_=par["wot"][l, 2 * dp:2 * dp + 2].rearrange(
                            "c p k m -> p c k m"))
                    out_wt_cache[(b, dp)] = wt
                wt = out_wt_cache[(b, dp)]
                tsl = bass.ts(b, 512)
                pq = pm.tile([P, 512], DT_F, name="pmt")
                for kt in range(KTILES):
                    nc.tensor.matmul(
                        pq, wt[:, di, kt, :], of[:, kt, tsl],
                        start=(kt == 0), stop=(kt == 7))
                if flags["bo_nz"]:
                    nc.scalar.activation(pq, pq, FX.Identity,
                                         bias=bov_t[:, l, dt:dt + 1])
                nc.vector.tensor_add(x[:, dt, tsl], pq, x[:, dt, tsl])

            def qkv_pair(b, cp):
                wt = wsm.tile([P, 2, KTILES, P], DT_H, name="wsm_t")
                nc.sync.dma_start(
                    out=wt,
                    in_=par["wqk_t"][l, 2 * cp:2 * cp + 2].rearrange(
                        "c p k m -> p c k m"))
                tsl = bass.ts(b, 512)
                for ci in range(2):
                    c = 2 * cp + ci
                    pq = pm.tile([P, 512], DT_F, name="pmt")
                    for kt in range(KTILES):
                        nc.tensor.matmul(pq, wt[:, ci, kt, :], h[:, kt, tsl],
                                         start=(kt == 0), stop=(kt == 7))
                    if c < DTILES:
                        nc.scalar.activation(qf_b[b][:, c, :], pq, FX.Identity)
                    else:
                        nc.scalar.activation(
                            kf_b[b][:, c - DTILES, :], pq, FX.Identity,
                            scale=float(DHEAD) ** -0.5)

            def v_group(b, nh, tth):
                tts = (2 * tth, 2 * tth + 1)
                pvs = [pm.tile([P, 512], DT_F, name="pmt") for _ in tts]
                for kp in range(KTILES // 2):
                    wv = wsm.tile([P, 2, 512], DT_H, name="wsm_t")
                    nc.sync.dma_start(
                        out=wv,
                        in_=par["wv_t"][l, nh, 2 * kp:2 * kp + 2].rearrange(
                            "k p n -> p k n"))
                    for ki in range(2):
                        kt = 2 * kp + ki
                        for ti, tt in enumerate(tts):
                            nc.tensor.matmul(
                                pvs[ti],
                                h[:, kt, b * 512 + tt * P:
                                  b * 512 + (tt + 1) * P],
                                wv[:, ki, :],
                                start=(kt == 0), stop=(kt == 7))
                for ti, tt in enumerate(tts):
                    nc.scalar.activation(
                        vt_b[b][:, tt, nh * 8:(nh + 1) * 8, 0:64],
                        pvs[ti].rearrange("p (hd d) -> p hd d", d=64),
                        FX.Identity)

            # ---- pipelined attention stages ----
            def attn_scores(b, hd):
                po = (hd % 2) * 64
                dt = hd // 2
                etr = etrp.tile([P, SEQT, 512], DT_H, name="etr")
                for jt in range(SEQT):
                    i0 = jt * P
                    sc = pm.tile([P, 512], DT_F, name="pmt")
                    nc.tensor.matmul(
                        sc[:, i0:],
                        kf_b[b][po:po + 64, dt, bass.ts(jt, P)],
                        qf_b[b][po:po + 64, dt, i0:],
                        start=True, stop=True)
                    ex = scrh.tile([P, 512], DT_H, name="lnt")
                    nc.scalar.activation(ex[:, i0:], sc[:, i0:], FX.Exp)
                    nc.vector.tensor_mul(
                        etr[:, jt, i0:], ex[:, i0:], et_sb[:, jt, i0:])
                return etr

            def attn_av(b, hd, etr):
                ot = pot.tile([P, 512], DT_F, name="ot")
                for jt in range(SEQT):
                    i0 = jt * P
                    nc.tensor.matmul(
                        ot[0:65, i0:],
                        vt_b[b][:, jt, hd, :],
                        etr[:, jt, i0:],
                        start=(jt == 0), stop=(jt == SEQT - 1))
                oden = odnp.tile([65, 512], DT_H, name="oden")
                nc.scalar.activation(oden, ot[0:65, :], FX.Identity)
                return oden

            def attn_tail(b, hd, oden):
                po = (hd % 2) * 64
                dt = hd // 2
                tsl = bass.ts(b, 512)
                pden = pm.tile([64, 512], DT_F, name="pmt")
                nc.tensor.matmul(pden, ones_bc[64:65, :], oden[64:65, :],
                                 start=True, stop=True)
                adrb = rdnp.tile([64, 512], DT_F, name="adrb")
                nc.vector.reciprocal_approx_fast(out=adrb, in_=pden)
                if po == 0:
                    nc.vector.tensor_mul(
                        of[0:64, dt, tsl], oden[0:64, :], adrb)
                else:
                    otmp = scrh.tile([64, 512], DT_H, name="lnt2")
                    nc.vector.tensor_mul(otmp, oden[0:64, :], adrb)
                    nc.sync.dma_start(out=of[64:128, dt, tsl], in_=otmp)

            def attn_pipeline(b, units):
                """Heads 0..15 of half b, one filler unit per head slot."""
                etr_d = {}
                oden_d = {}
                ui = [0]

                def pump():
                    if ui[0] < len(units):
                        units[ui[0]]()
                        ui[0] += 1

                for hd in range(HEADS):
                    if hd >= 1:
                        oden_d[hd - 1] = attn_av(b, hd - 1, etr_d.pop(hd - 1))
                    etr_d[hd] = attn_scores(b, hd)
                    pump()
                    if hd >= 2:
                        attn_tail(b, hd - 2, oden_d.pop(hd - 2))
                oden_d[15] = attn_av(b, 15, etr_d.pop(15))
                attn_tail(b, 14, oden_d.pop(14))
                while ui[0] < len(units):
                    pump()
                attn_tail(b, 15, oden_d.pop(15))

            out_wt_cache = {}

            def out_pair(b, dp):
                wt = wsm.tile([P, 2, KTILES, P], DT_H, name="wsm_t")
                nc.sync.dma_start(
                    out=wt,
                    in_=par["wot"][l, 2 * dp:2 * dp + 2].rearrange(
                        "c p k m -> p c k m"))
                tsl = bass.ts(b, 512)
                for di in range(2):
                    dt = 2 * dp + di
                    pq = pm.tile([P, 512], DT_F, name="pmt")
                    for kt in range(KTILES):
                        nc.tensor.matmul(
                            pq, wt[:, di, kt, :], of[:, kt, tsl],
                            start=(kt == 0), stop=(kt == 7))
                    if flags["bo_nz"]:
                        nc.scalar.activation(pq, pq, FX.Identity,
                                             bias=bov_t[:, l, dt:dt + 1])
                    nc.vector.tensor_add(x[:, dt, tsl], pq, x[:, dt, tsl])

            # ---- phase B: qkv b0 (+ LNa th1 bcast mid-way) + v(b0) ----
            with nc.named_scope(f"L{l}_qkv0"):
                if l == 0:
                    for d in range(DTILES):
                        norm_a(0, d)
                for cp in range(8):
                    qkv_pair(0, cp)
                    if cp == 3:
                        planes_a1 = ln_bcast(rows_a1)
                        for d in range(DTILES):
                            norm_a(1, d)
                for nh in range(2):
                    for tth in range(2):
                        v_group(0, nh, tth)

            # ---- att0: heads b0, fillers = qkv(b1) c-tiles + v(b1) ----
            qf1 = qkvp.tile([P, DTILES, 512], DT_H, name="qf")
            kf1 = qkvp.tile([P, DTILES, 512], DT_H, name="kf")
            vt41 = qkvp.tile([P, SEQT, HEADS, 65], DT_H, name="vt")
            nc.vector.memset(vt41[:, :, :, 64:65].bitcast(mybir.dt.uint16),
                             0x3F80)
            qf_b[1], kf_b[1], vt_b[1] = qf1, kf1, vt41

            units0 = (
                [lambda c=c: qkv_half(1, c)
                 for c in (0, 1, 2, 3, 4, 5, 8, 9, 10, 11, 12, 13)]
                + [lambda nh=nh, tth=tth: v_group(1, nh, tth)
                   for nh in range(2) for tth in range(2)]
            )
            with nc.named_scope(f"L{l}_att0"):
                attn_pipeline(0, units0)

            # ---- att1: heads b1, fillers = deferred qkv(b1) + out(b0) ----
            units1 = (
                [lambda c=c: qkv_half(1, c) for c in (6, 14, 7, 15)]
                + [lambda dt=dt: out_half(0, dt) for dt in range(DTILES)]
            )
            with nc.named_scope(f"L{l}_att1"):
                attn_pipeline(1, units1)

            # ---- out tail: LNf stats + out b1 ----
            with nc.named_scope(f"L{l}_out1"):
                psf0 = ln_stats(0)
                rows_f0 = ln_tail(*psf0)
                for dp in range(2):
                    out_pair(1, dp)
                planes_f0 = ln_bcast(rows_f0)
                for dp in range(2, 4):
                    out_pair(1, dp)
                psf1 = ln_stats(1)
                rows_f1 = ln_tail(*psf1)

            h2 = h2p.tile([P, DTILES, 512], DT_H, name="h2")
            h1r = h1p.tile([P, MTILES, 512], DT_H, name="h1r")

            def norm_f(th, d, planes):
                ln_norm(th, d, planes, h2[:, d, :],
                        lnfg_t[:, l, d:d + 1], lnfb_t[:, l, d:d + 1], gb_f)

            def w1_pass(th):
                for mp in range(MTILES // 2):
                    wt = wsm.tile([P, 2, KTILES, P], DT_H, name="wsm_t")
                    nc.sync.dma_start(
                        out=wt,
                        in_=par["w1t"][l, 2 * mp:2 * mp + 2].rearrange(
                            "c p k m -> p c k m"))
                    for mi in range(2):
                        mt = 2 * mp + mi
                        pq = pm.tile([P, 512], DT_F, name="pmt")
                        for kt in range(KTILES):
                            nc.tensor.matmul(
                                pq, wt[:, mi, kt, :], h2[:, kt, :],
                                start=(kt == 0), stop=(kt == 7))
                        nc.scalar.activation(
                            h1r[:, mt, :], pq, FX.Gelu,
                            bias=b1v_t[:, l, mt:mt + 1], scale=1.0)

            def w2_pass(th):
                tsl = bass.ts(th, 512)
                for dt in range(DTILES):
                    pq = pm.tile([P, 512], DT_F, name="pmt")
                    for kh in range(2):
                        wt = wsm.tile([P, 16, P], DT_H, name="wsm_t")
                        nc.sync.dma_start(out=wt, in_=par["w2t"][l, dt, kh])
                        for k2 in range(16):
                            kt = kh * 16 + k2
                            nc.tensor.matmul(
                                pq, wt[:, k2, :], h1r[:, kt, :],
                                start=(kt == 0), stop=(kt == 31))
                    if flags["b2_nz"]:
                        nc.scalar.activation(pq, pq, FX.Identity,
                                             bias=b2v_t[:, l, dt:dt + 1])
                    nc.vector.tensor_add(x[:, dt, tsl], pq, x[:, dt, tsl])

            with nc.named_scope(f"L{l}_ffn"):
                for d in range(DTILES):
                    norm_f(0, d, planes_f0)
                w1_pass(0)
                pf1 = ln_bcast(rows_f1)
                for d in range(DTILES):
                    norm_f(1, d, pf1)
                w2_pass(0)
                if l < DEPTH - 1:
                    psn0 = ln_stats(0)
                    rows_n0 = ln_tail(*psn0)
                w1_pass(1)
                if l < DEPTH - 1:
                    pn0 = ln_bcast(rows_n0)
                    for d in range(DTILES):
                        norm_a(0, d, planes=pn0, lidx=l + 1)
                w2_pass(1)
                if l < DEPTH - 1:
                    psn1 = ln_stats(1)
                    carry_rows_a1 = ln_tail(*psn1)

        # =================== head ===================
        for th in range(2):
            ps = ln_stats(th)
            rows_o = ln_tail(*ps)
            pl_o = ln_bcast(rows_o)
            for d in range(DTILES):
                ln_norm(th, d, pl_o, h[:, d, bass.ts(th, 512)],
                        lnog_t[:, d:d + 1], lnob_t[:, d:d + 1],
                        flags["gb_o"])
        wp3 = par["wproj"].rearrange("(ko p) m -> p ko m", p=P)
        wt = wsm.tile([P, KTILES, NCLS], DT_H, name="wsm_t")
        nc.sync.dma_start(out=wt, in_=wp3)
        out_sb = h2p.tile([NCLS, TOK], DT_F, name="h2")
        for th in range(2):
            pq = pot.tile([P, 512], DT_F, name="ot")
            for kt in range(KTILES):
                nc.tensor.matmul(pq[0:NCLS, :], wt[:, kt, :],
                                 h[:, kt, bass.ts(th, 512)],
                                 start=(kt == 0), stop=(kt == 7))
            nc.scalar.activation(out_sb[:, bass.ts(th, 512)],
                                 pq[0:NCLS, :],
                                 FX.Identity, bias=bproj_t)
        nc.sync.dma_start(out=par["out"][:, :], in_=out_sb)


# ============================================================
# host side
# ============================================================

_NC_CACHE = None
_NC_FLAGS = None


def _bf16(a):
    import ml_dtypes
    return np.ascontiguousarray(a.astype(ml_dtypes.bfloat16))


def _pack_qk(w):      # [D, DIM, 3072] -> [D, 16, P, 8, P]
    v = w[:, :, :2048].reshape(DEPTH, 8, P, 16, P).transpose(0, 3, 2, 1, 4)
    return _bf16(v)


def _pack_v(w):       # -> [D, 2, 8, P, 512]
    v = w[:, :, 2048:].reshape(DEPTH, 8, P, 2, 512).transpose(0, 3, 1, 2, 4)
    return _bf16(v)


def _pack_kxm(w):     # [D, K, M] -> [D, M//P, P, K//P, P]
    D, K, M = w.shape
    v = w.reshape(D, K // P, P, M // P, P).transpose(0, 3, 2, 1, 4)
    return _bf16(v)


def _pack_w2(w):      # [D, 4096, 1024] -> [D, 8, 2, P, 16, P]
    v = w.reshape(DEPTH, 2, 16, P, 8, P).transpose(0, 4, 1, 3, 2, 5)
    return _bf16(v)


def _host_band():
    tt = np.arange(KSIZE, dtype=np.float64)
    kern = np.exp(-0.5 * ((tt - (KSIZE - 1) / 2.0) / SIGMA) ** 2)
    kern = (kern / kern.sum()).astype(np.float32)
    pad_l = (KSIZE - 1) // 2  # 9
    nt = T // P
    bandc = np.zeros((nt, 3, P, P), dtype=np.float32)
    for ct in range(nt):
        for s in range(3):
            kt = ct - 1 + s
            if not (0 <= kt < nt):
                continue
            rows = np.arange(kt * P, (kt + 1) * P)
            cols = np.arange(ct * P, (ct + 1) * P)
            d = rows[:, None] - cols[None, :] + pad_l
            m = (d >= 0) & (d < KSIZE)
            blk = np.zeros((P, P), np.float32)
            blk[m] = kern[d[m]]
            bandc[ct, s] = blk
    return bandc


def _host_etab(rel_tab):
    i = np.arange(SEQ)
    j = i[:, None]
    rel = np.clip(i[None, :] - j, -(MAXREL - 1), MAXREL - 1) + MAXREL - 1
    et = np.zeros((DEPTH, SEQ, SEQ), dtype=np.float32)
    for l in range(DEPTH):
        e = np.exp(rel_tab[l][rel])
        e[j > i[None, :]] = 0.0
        et[l] = e
    return _bf16(et.reshape(DEPTH, SEQT, P, SEQ))


def kernel(**inputs):
    global _NC_CACHE, _NC_FLAGS

    f32 = lambda a: np.ascontiguousarray(np.asarray(a, dtype=np.float32))
    z = lambda a: bool(np.any(np.asarray(a) != 0))
    one = lambda a: bool(np.all(np.asarray(a) == 1.0))
    flags = {
        "gb_a": (not one(inputs["ln_a_g"])) or z(inputs["ln_a_b"]),
        "gb_f": (not one(inputs["ln_f_g"])) or z(inputs["ln_f_b"]),
        "gb_o": (not one(inputs["ln_o_g"])) or z(inputs["ln_o_b"]),
        "bo_nz": z(inputs["bo"]),
        "b2_nz": z(inputs["b2"]),
    }
    if _NC_CACHE is None or _NC_FLAGS != flags:
        _NC_CACHE = build_nc(flags)
        _NC_FLAGS = dict(flags)
    nc = _NC_CACHE

    def vx(a):       # [width] -> [P, width//P]  (partition-major)
        a = f32(a)
        return np.ascontiguousarray(a.reshape(-1, P).T)

    def vxl(a):      # [L, width] -> [P, L, width//P]
        a = f32(a)
        L = a.shape[0]
        return np.ascontiguousarray(a.reshape(L, -1, P).transpose(2, 0, 1))

    shared = {
        "band": _host_band(),
        "etab": _host_etab(f32(inputs["rel_tab"])),
        "wpe": f32(inputs["W_pe"]),
        "wqk_t": _pack_qk(f32(inputs["Wqkv"])),
        "wv_t": _pack_v(f32(inputs["Wqkv"])),
        "wot": _pack_kxm(f32(inputs["Wo"])),
        "w1t": _pack_kxm(f32(inputs["W1"])),
        "w2t": _pack_w2(f32(inputs["W2"])),
        "wproj": _bf16(f32(inputs["Wproj"])),
        "lnp1g": vx(inputs["ln_p1_g"]), "lnp1b": vx(inputs["ln_p1_b"]),
        "bpe": vx(inputs["b_pe"]),
        "lnp2g": vx(inputs["ln_p2_g"]), "lnp2b": vx(inputs["ln_p2_b"]),
        "lnag": vxl(inputs["ln_a_g"]), "lnab": vxl(inputs["ln_a_b"]),
        "lnfg": vxl(inputs["ln_f_g"]), "lnfb": vxl(inputs["ln_f_b"]),
        "bov": vxl(inputs["bo"]), "b1v": vxl(inputs["b1"]),
        "b2v": vxl(inputs["b2"]),
        "lnog": vx(inputs["ln_o_g"]), "lnob": vx(inputs["ln_o_b"]),
        "bprojv": f32(inputs["bproj"]),
    }
    xfull = f32(inputs["neuralInput"])
    in_maps = []
    for c in range(NCORES):
        m = dict(shared)
        m["xin"] = np.ascontiguousarray(xfull[c * BPC:(c + 1) * BPC])
        in_maps.append(m)

    import os
    trace = bool(os.environ.get("BIT_TRACE"))
    res = run_bass_kernel_spmd(nc, in_maps, list(range(NCORES)), trace=trace)
    if trace:
        globals()["LAST_RESULT"] = res
    outs = []
    for c in range(NCORES):
        o = res.results[c]["out"]              # [NCLS, TOK]
        o = o.reshape(NCLS, BPC, SEQ).transpose(1, 2, 0)
        outs.append(o)
    return np.concatenate(outs, axis=0).astype(np.float32)


# revision 28
# speedup vs baseline: 1.2264x; 1.0044x over previous
"""Trainium2 Bass kernel for nn_BiT_Phoneme (dense transformer), v3.

Data-parallel: 16 batch elems / 8 cores = 2 per core. Feature-major
activations ([dim on partitions, tokens on free]); residual x in bf16.
v3 keeps the PE warm through attention (HAM was throttling ~75% of the
attention region in v2):
  - attention heads run as a 3-stage software pipeline per half:
    scores(h) / attnV(h-1) / den-tail(h-2), with one dense filler unit
    (qkv c-tile, v group, or out-proj dtile) per head slot so the PE
    never idles long enough to re-throttle.
  - den tail: reciprocal on the [1,512] den row, then K=1 broadcast
    matmul; of-mul reads the broadcast straight from PSUM (drops the
    dsb copy + [64,512] reciprocal of v2).
  - oden evac moved to ACT identity (DVE was the attention bottleneck).
  - LN stats use DVE add-trees + 2 matmuls instead of 16 stat matmuls.
  - out-proj weights double-pumped: b0 as att1 fillers, b1 after.
"""

import numpy as np

import concourse.bass as bass
import concourse.mybir as mybir
import concourse.tile as tile
from concourse import bacc
from concourse.bass_utils import run_bass_kernel_spmd

B, T, F = 16, 2048, 256
PH = 4
PATCH = 1024
DIM = 1024
DEPTH = 6
HEADS, DHEAD = 16, 64
INNER = 1024
MLP = 4096
NCLS = 41
MAXREL = 200
KSIZE, SIGMA = 20, 2.0
EPS = 1e-5
SEQ = T // PH              # 512
NCORES = 8
BPC = B // NCORES          # 2
TOK = BPC * SEQ            # 1024
P = 128

DT_R = mybir.dt.float32r
DT_F = mybir.dt.float32
DT_H = mybir.dt.bfloat16
FX = mybir.ActivationFunctionType
OP = mybir.AluOpType

DTILES = DIM // P          # 8
KTILES = DIM // P          # 8
MTILES = MLP // P          # 32
SEQT = SEQ // P            # 4


def build_nc(flags):
    nc = bacc.Bacc(None, target_bir_lowering=False)

    par = {}
    def dp(name, shape, dtype, is_out=False):
        par[name] = nc.declare_dram_parameter(name, list(shape), dtype, isOutput=is_out)
        return par[name]

    dp("xin", (BPC, T, F), DT_R)
    dp("band", (T // P, 3, P, P), DT_R)
    dp("etab", (DEPTH, SEQT, P, SEQ), DT_H)
    dp("wpe", (PATCH, DIM), DT_R)
    dp("wqk_t", (DEPTH, 16, P, KTILES, P), DT_H)
    dp("wv_t", (DEPTH, 2, KTILES, P, 512), DT_H)
    dp("wot", (DEPTH, DTILES, P, KTILES, P), DT_H)
    dp("w1t", (DEPTH, MTILES, P, KTILES, P), DT_H)
    dp("w2t", (DEPTH, DTILES, 2, P, 16, P), DT_H)
    dp("wproj", (DIM, NCLS), DT_H)
    # vectors arrive pre-transposed to [P, (L,) width//P] so the DMA is
    # one contiguous run per partition instead of an element gather
    for nm, width, L in [("lnp1g", PATCH, 0), ("lnp1b", PATCH, 0),
                         ("bpe", DIM, 0), ("lnp2g", DIM, 0),
                         ("lnp2b", DIM, 0),
                         ("lnag", DIM, DEPTH), ("lnab", DIM, DEPTH),
                         ("lnfg", DIM, DEPTH), ("lnfb", DIM, DEPTH),
                         ("bov", DIM, DEPTH), ("b1v", MLP, DEPTH),
                         ("b2v", DIM, DEPTH), ("lnog", DIM, 0),
                         ("lnob", DIM, 0)]:
        shp = (P, L, width // P) if L else (P, width // P)
        dp(nm, shp, DT_F)
    dp("bprojv", (NCLS,), DT_F)
    dp("out", (NCLS, TOK), DT_F, is_out=True)

    with tile.TileContext(nc) as tc:
        _emit(nc, tc, par, flags)
    nc.compile()
    return nc


def _emit(nc, tc, par, flags):
    import contextlib
    ctx = contextlib.ExitStack()
    with ctx:
        const = ctx.enter_context(tc.tile_pool(name="const", bufs=1))
        xpool = ctx.enter_context(tc.tile_pool(name="xpool", bufs=1))
        wsm = ctx.enter_context(tc.tile_pool(name="wsm", bufs=5))
        scr = ctx.enter_context(tc.tile_pool(name="scr", bufs=4))
        scrh = ctx.enter_context(tc.tile_pool(name="scrh", bufs=2))
        stp = ctx.enter_context(tc.tile_pool(name="stp", bufs=2))
        plh = ctx.enter_context(tc.tile_pool(name="plh", bufs=4))
        rowp = ctx.enter_context(tc.tile_pool(name="rowp", bufs=2))
        odnp = ctx.enter_context(tc.tile_pool(name="odnp", bufs=2))
        rdnp = ctx.enter_context(tc.tile_pool(name="rdnp", bufs=2))
        pm = ctx.enter_context(tc.tile_pool(name="pm", bufs=4, space="PSUM"))
        pot = ctx.enter_context(tc.tile_pool(name="pot", bufs=2, space="PSUM"))
        pst = ctx.enter_context(tc.tile_pool(name="pst", bufs=1, space="PSUM"))

        ones_r = const.tile([P, 1], DT_R, name="ones_r")
        nc.vector.memset(ones_r.bitcast(mybir.dt.uint32), 0x3F800000)
        ones_h = const.tile([P, 1], DT_H, name="ones_h")
        nc.vector.memset(ones_h.bitcast(mybir.dt.uint16), 0x3F80)
        ones_row = const.tile([1, P], DT_R, name="ones_row")
        nc.vector.memset(ones_row.bitcast(mybir.dt.uint32), 0x3F800000)
        ones_bc = const.tile([P, 64], DT_H, name="ones_bc")
        nc.vector.memset(ones_bc.bitcast(mybir.dt.uint16), 0x3F80)
        epst = const.tile([1, 1], DT_F, name="epst")
        nc.vector.memset(epst, EPS)

        def load_vec(nm, width):
            d = par[nm]
            tl = const.tile(list(d.shape), DT_F, name=nm + "_t")
            nc.sync.dma_start(out=tl, in_=d[tuple(slice(None) for _ in d.shape)])
            return tl

        lnp1g_t = load_vec("lnp1g", PATCH)
        lnp1b_t = load_vec("lnp1b", PATCH)
        bpe_t = load_vec("bpe", DIM)
        lnp2g_t = load_vec("lnp2g", DIM)
        lnp2b_t = load_vec("lnp2b", DIM)
        lnag_t = load_vec("lnag", DIM)
        lnab_t = load_vec("lnab", DIM)
        lnfg_t = load_vec("lnfg", DIM)
        lnfb_t = load_vec("lnfb", DIM)
        bov_t = load_vec("bov", DIM)
        b1v_t = load_vec("b1v", MLP)
        b2v_t = load_vec("b2v", DIM)
        lnog_t = load_vec("lnog", DIM)
        lnob_t = load_vec("lnob", DIM)
        bproj_t = const.tile([NCLS, 1], DT_F, name="bproj_t")
        nc.sync.dma_start(out=bproj_t,
                          in_=par["bprojv"].rearrange("(p o) -> p o", o=1))

        x = xpool.tile([P, DTILES, TOK], DT_H, name="x")

        # ---------- embedding layernorms (split stats / tail / finish) ----
        def emb_stats(views, use_ot):
            """16 accumulating stat matmuls over 8 f32r [P,512] views."""
            if use_ot:
                ps0 = pot.tile([1, 512], DT_F, name="ot")
                ps1 = pot.tile([1, 512], DT_F, name="ot")
            else:
                ps0 = pst.tile([1, 512], DT_F, name="ps0")
                ps1 = pst.tile([1, 512], DT_F, name="ps1")
            for d in range(8):
                v = views(d)
                sq = scr.tile([P, 512], DT_R, name="scr")
                nc.vector.tensor_mul(sq, v, v)
                nc.tensor.matmul(ps0, ones_r, v, start=(d == 0), stop=(d == 7))
                nc.tensor.matmul(ps1, ones_r, sq, start=(d == 0), stop=(d == 7))
            return ps0, ps1

        def emb_finish(views, dst_fn, rows, g_fn, b_fn):
            """bcast (r, mu*r) planes and normalize 8 views into dst."""
            p1, p2 = ln_bcast(rows)
            for d in range(8):
                t = scr.tile([P, 512], DT_F, name="scr")
                nc.vector.tensor_mul(t, views(d), p1)
                nc.vector.tensor_sub(t, t, p2)
                nc.vector.tensor_scalar(dst_fn(d), t, g_fn(d), b_fn(d),
                                        OP.mult, OP.add)

        def ln_tail(ps0, ps1):
            rows = rowp.tile([1, 2, 512], DT_R, name="rows")
            a = scr.tile([1, 512], DT_F, name="scr")
            v = scr.tile([1, 512], DT_F, name="scr")
            m2 = scr.tile([1, 512], DT_F, name="scr")
            nc.vector.tensor_scalar(a, ps0, 1.0 / DIM, None, OP.mult)
            nc.vector.tensor_scalar(v, ps1, 1.0 / DIM, None, OP.mult)
            nc.vector.tensor_mul(m2, a, a)
            nc.vector.tensor_sub(v, v, m2)
            nc.scalar.activation(v, v, FX.Sqrt, bias=epst, scale=1.0)
            rv = scr.tile([1, 512], DT_F, name="scr")
            nc.vector.reciprocal_approx_fast(out=rv, in_=v)
            nc.vector.tensor_copy(rows[:, 0, :], rv)
            nc.vector.tensor_mul(rows[:, 1, :], a, rv)
            return rows

        def ln_bcast(rows):
            out = []
            for s in range(2):
                pp = pm.tile([P, 512], DT_F, name="pmt")
                nc.tensor.matmul(pp, ones_row, rows[:, s, :],
                                 start=True, stop=True)
                psb = plh.tile([P, 512], DT_H, name="plh")
                nc.scalar.activation(psb, pp, FX.Identity)
                out.append(psb)
            return tuple(out)

        # =================== embedding ===================
        xin, band = par["xin"], par["band"]
        with (
            tc.tile_pool(name="sfp", bufs=1) as sfp,
            tc.tile_pool(name="pnp", bufs=1) as pnp,
            tc.tile_pool(name="x0p", bufs=1) as x0p,
            tc.tile_pool(name="xap", bufs=16) as xap,
        ):
            x0 = x0p.tile([P, DTILES, TOK], DT_R, name="x0")
            sf4 = sfp.tile([P, BPC, 2, T], DT_R, name="sf4")
            bfh = [(b, fh) for b in range(BPC) for fh in range(2)]
            xa_cache = {}

            def get_xa(b, kt, fh):
                key = (b, kt, fh)
                if key not in xa_cache:
                    t = xap.tile([P, P], DT_R, name="xa")
                    nc.sync.dma_start(
                        out=t, in_=xin[b, bass.ts(kt, P), bass.ts(fh, P)])
                    xa_cache[key] = t
                return xa_cache[key]

            # prefetch the first few xa tiles before anything else queues
            for kt in range(2):
                for (b, fh) in bfh:
                    get_xa(b, kt, fh)

            for g4 in range(T // 512):
                pgs = {}
                for i, (b, fh) in enumerate(bfh):
                    pool = pm if i < 2 else pot
                    pgs[(b, fh)] = pool.tile([P, 512], DT_F,
                                             name="pmt" if i < 2 else "ot")
                for q in range(4):
                    ct = g4 * 4 + q
                    bt = wsm.tile([P, 3, P], DT_R, name="wsm_t")
                    nc.sync.dma_start(
                        out=bt, in_=band[ct].rearrange("s p q -> p s q"))
                    svals = [s for s in range(3)
                             if 0 <= ct - 1 + s < T // P]
                    for (b, fh) in bfh:
                        for si, s in enumerate(svals):
                            kt = ct - 1 + s
                            xa = get_xa(b, kt, fh)
                            nc.tensor.matmul(
                                pgs[(b, fh)][:, bass.ts(q, P)],
                                xa,
                                bt[:, s, :],
                                start=(q == 0 and si == 0),
                                stop=(q == 3 and si == len(svals) - 1))
                for (b, fh) in bfh:
                    nc.vector.tensor_copy(
                        sf4[:, b, fh, bass.ts(g4, 512)], pgs[(b, fh)])

            pn = pnp.tile([P, BPC, 8, 512], DT_R, name="pn")

            def mk_pview(b):
                def pview(pt):
                    i, fh = pt // 2, pt % 2
                    return sf4[:, b, fh, :].rearrange(
                        "p (s four) -> p four s", four=PH)[:, i, :]
                return pview

            pviews = [mk_pview(b) for b in range(BPC)]
            # b1 stats cover b0's tail chain; finishes pipeline likewise
            s0 = emb_stats(pviews[0], False)
            s1 = emb_stats(pviews[1], True)
            r0 = ln_tail(*s0)
            r1 = ln_tail(*s1)
            emb_finish(pviews[0], lambda d: pn[:, 0, d, :], r0,
                       lambda d: lnp1g_t[:, d:d + 1],
                       lambda d: lnp1b_t[:, d:d + 1])
            emb_finish(pviews[1], lambda d: pn[:, 1, d, :], r1,
                       lambda d: lnp1g_t[:, d:d + 1],
                       lambda d: lnp1b_t[:, d:d + 1])

            # wpe matmuls with x0-LN stats folded into the stream
            psxS = [pst.tile([1, 512], DT_F, name="ps0"),
                    pot.tile([1, 512], DT_F, name="ot")]
            psxQ = [pst.tile([1, 512], DT_F, name="ps1"),
                    pot.tile([1, 512], DT_F, name="ot")]
            for dt in range(DTILES):
                wt = wsm.tile([P, KTILES, P], DT_R, name="wsm_t")
                nc.sync.dma_start(
                    out=wt,
                    in_=par["wpe"].rearrange("(ko p) m -> p ko m", p=P)[
                        :, :, bass.ts(dt, P)])
                for b in range(BPC):
                    pq = pm.tile([P, 512], DT_F, name="pmt")
                    for kt in range(KTILES):
                        nc.tensor.matmul(pq, wt[:, kt, :], pn[:, b, kt, :],
                                         start=(kt == 0), stop=(kt == 7))
                    v = x0[:, dt, bass.ts(b, 512)]
                    nc.vector.tensor_scalar(
                        v, pq, bpe_t[:, dt:dt + 1], None, OP.add)
                    sq = scr.tile([P, 512], DT_R, name="scr")
                    nc.vector.tensor_mul(sq, v, v)
                    nc.tensor.matmul(psxS[b], ones_r, v,
                                     start=(dt == 0), stop=(dt == DTILES - 1))
                    nc.tensor.matmul(psxQ[b], ones_r, sq,
                                     start=(dt == 0), stop=(dt == DTILES - 1))
            rows_x = [ln_tail(psxS[th], psxQ[th]) for th in range(2)]
            for th in range(2):
                emb_finish(lambda d, th=th: x0[:, d, bass.ts(th, 512)],
                           lambda d, th=th: x[:, d, bass.ts(th, 512)],
                           rows_x[th],
                           lambda d: lnp2g_t[:, d:d + 1],
                           lambda d: lnp2b_t[:, d:d + 1])

        # main pools (opened after embedding scratch closes)
        hpool = ctx.enter_context(tc.tile_pool(name="hpool", bufs=1))
        h2p = ctx.enter_context(tc.tile_pool(name="h2p", bufs=1))
        qkvp = ctx.enter_context(tc.tile_pool(name="qkvp", bufs=2))
        ofp = ctx.enter_context(tc.tile_pool(name="ofp", bufs=1))
        h1p = ctx.enter_context(tc.tile_pool(name="h1p", bufs=1))
        etrp = ctx.enter_context(tc.tile_pool(name="etrp", bufs=2))
        etp = ctx.enter_context(tc.tile_pool(name="etp", bufs=1))

        h = hpool.tile([P, DTILES, TOK], DT_H, name="h")
        of = ofp.tile([P, DTILES, TOK], DT_H, name="of")

        # ---------- split layernorm helpers (main layers) ----------
        def ln_stats(th):
            """DVE add-chains for sum/sumsq + 2 partition-reduce matmuls."""
            sl = bass.ts(th, 512)
            ps0 = pst.tile([1, 512], DT_F, name="ps0")
            ps1 = pst.tile([1, 512], DT_F, name="ps1")

            flip = [0]

            def acc_tile():
                t = stp.tile([P, 512], DT_H,
                             name=("ta" if flip[0] else "tb"))
                flip[0] ^= 1
                return t

            sacc = x[:, 0, sl]
            for d in range(1, DTILES):
                t = acc_tile()
                nc.vector.tensor_add(t, sacc, x[:, d, sl])
                sacc = t
            nc.tensor.matmul(ps0, ones_h, sacc, start=True, stop=True)
            qacc = None
            for d in range(DTILES):
                q = stp.tile([P, 512], DT_H, name="sq")
                nc.vector.tensor_mul(q, x[:, d, sl], x[:, d, sl])
                if qacc is None:
                    qacc = q
                else:
                    t = acc_tile()
                    nc.vector.tensor_add(t, qacc, q)
                    qacc = t
            nc.tensor.matmul(ps1, ones_h, qacc, start=True, stop=True)
            return ps0, ps1

        def ln_norm(th, d, planes, dst, g_ap, b_ap, apply_gb):
            sl = bass.ts(th, 512)
            p1, p2 = planes
            t = scrh.tile([P, 512], DT_H, name="lnt")
            nc.vector.tensor_mul(t, x[:, d, sl], p1)
            if apply_gb:
                t2 = scrh.tile([P, 512], DT_H, name="lnt2")
                nc.vector.tensor_sub(t2, t, p2)
                nc.vector.tensor_scalar(dst, t2, g_ap, b_ap, OP.mult, OP.add)
            else:
                nc.vector.tensor_sub(dst, t, p2)

        # =================== transformer layers ===================
        carry_rows_a1 = None
        for l in range(DEPTH):
            gb_a = flags["gb_a"]
            gb_f = flags["gb_f"]

            et_sb = etp.tile([P, SEQT, SEQ], DT_H, name="et_sb")
            nc.sync.dma_start(out=et_sb,
                              in_=par["etab"][l].rearrange("jt p i -> p jt i"))

            if l == 0:
                with nc.named_scope(f"L{l}_lna"):
                    psa0 = ln_stats(0)
                    rows_a0 = ln_tail(*psa0)
                    psa1 = ln_stats(1)
                    rows_a1 = ln_tail(*psa1)
                    planes_a0 = ln_bcast(rows_a0)
            else:
                rows_a1 = carry_rows_a1

            qf = qkvp.tile([P, DTILES, 512], DT_H, name="qf")
            kf = qkvp.tile([P, DTILES, 512], DT_H, name="kf")
            vt4 = qkvp.tile([P, SEQT, HEADS, 65], DT_H, name="vt")
            nc.vector.memset(vt4[:, :, :, 64:65].bitcast(mybir.dt.uint16),
                             0x3F80)
            qf_b = [qf, None]
            kf_b = [kf, None]
            vt_b = [vt4, None]

            def norm_a(th, d, planes=None, lidx=None):
                li = l if lidx is None else lidx
                pl_ = planes if planes is not None else (
                    planes_a0 if th == 0 else planes_a1)
                ln_norm(th, d, pl_, h[:, d, bass.ts(th, 512)],
                        lnag_t[:, li, d:d + 1], lnab_t[:, li, d:d + 1], gb_a)

            qkv_wt_cache = {}

            def qkv_half(b, c):
                cp, ci = c // 2, c % 2
                if ci == 0:
                    wt = wsm.tile([P, 2, KTILES, P], DT_H, name="wsm_t")
                    nc.sync.dma_start(
                        out=wt,
                        in_=par["wqk_t"][l, 2 * cp:2 * cp + 2].rearrange(
                            "c p k m -> p c k m"))
                    qkv_wt_cache[(b, cp)] = wt
                wt = qkv_wt_cache[(b, cp)]
                tsl = bass.ts(b, 512)
                pq = pm.tile([P, 512], DT_F, name="pmt")
                for kt in range(KTILES):
                    nc.tensor.matmul(pq, wt[:, ci, kt, :], h[:, kt, tsl],
                                     start=(kt == 0), stop=(kt == 7))
                if c < DTILES:
                    nc.vector.tensor_copy(qf_b[b][:, c, :], pq)
                else:
                    nc.vector.tensor_scalar(
                        kf_b[b][:, c - DTILES, :], pq,
                        float(DHEAD) ** -0.5, None, OP.mult)

            def out_half(b, dt):
                dp, di = dt // 2, dt % 2
                if di == 0:
                    wt = wsm.tile([P, 2, KTILES, P], DT_H, name="wsm_t")
                    nc.sync.dma_start(
                        out=wt,
                        in_=par["wot"][l, 2 * dp:2 * dp + 2].rearrange(
                            "c p k m -> p c k m"))
                    out_wt_cache[(b, dp)] = wt
                wt = out_wt_cache[(b, dp)]
                tsl = bass.ts(b, 512)
                pq = pm.tile([P, 512], DT_F, name="pmt")
                for kt in range(KTILES):
                    nc.tensor.matmul(
                        pq, wt[:, di, kt, :], of[:, kt, tsl],
                        start=(kt == 0), stop=(kt == 7))
                if flags["bo_nz"]:
                    nc.scalar.activation(pq, pq, FX.Identity,
                                         bias=bov_t[:, l, dt:dt + 1])
                nc.vector.tensor_add(x[:, dt, tsl], pq, x[:, dt, tsl])

            def qkv_pair(b, cp):
                wt = wsm.tile([P, 2, KTILES, P], DT_H, name="wsm_t")
                nc.sync.dma_start(
                    out=wt,
                    in_=par["wqk_t"][l, 2 * cp:2 * cp + 2].rearrange(
                        "c p k m -> p c k m"))
                tsl = bass.ts(b, 512)
                for ci in range(2):
                    c = 2 * cp + ci
                    pq = pm.tile([P, 512], DT_F, name="pmt")
                    for kt in range(KTILES):
                        nc.tensor.matmul(pq, wt[:, ci, kt, :], h[:, kt, tsl],
                                         start=(kt == 0), stop=(kt == 7))
                    if c < DTILES:
                        nc.scalar.activation(qf_b[b][:, c, :], pq, FX.Identity)
                    else:
                        nc.scalar.activation(
                            kf_b[b][:, c - DTILES, :], pq, FX.Identity,
                            scale=float(DHEAD) ** -0.5)

            def v_group(b, nh, tth):
                tts = (2 * tth, 2 * tth + 1)
                pvs = [pm.tile([P, 512], DT_F, name="pmt") for _ in tts]
                for kp in range(KTILES // 2):
                    wv = wsm.tile([P, 2, 512], DT_H, name="wsm_t")
                    nc.sync.dma_start(
                        out=wv,
                        in_=par["wv_t"][l, nh, 2 * kp:2 * kp + 2].rearrange(
                            "k p n -> p k n"))
                    for ki in range(2):
                        kt = 2 * kp + ki
                        for ti, tt in enumerate(tts):
                            nc.tensor.matmul(
                                pvs[ti],
                                h[:, kt, b * 512 + tt * P:
                                  b * 512 + (tt + 1) * P],
                                wv[:, ki, :],
                                start=(kt == 0), stop=(kt == 7))
                for ti, tt in enumerate(tts):
                    nc.scalar.activation(
                        vt_b[b][:, tt, nh * 8:(nh + 1) * 8, 0:64],
                        pvs[ti].rearrange("p (hd d) -> p hd d", d=64),
                        FX.Identity)

            # ---- pipelined attention stages ----
            def attn_scores(b, hd):
                po = (hd % 2) * 64
                dt = hd // 2
                etr = etrp.tile([P, SEQT, 512], DT_H, name="etr")
                for jt in range(SEQT):
                    i0 = jt * P
                    sc = pm.tile([P, 512], DT_F, name="pmt")
                    nc.tensor.matmul(
                        sc[:, i0:],
                        kf_b[b][po:po + 64, dt, bass.ts(jt, P)],
                        qf_b[b][po:po + 64, dt, i0:],
                        start=True, stop=True)
                    ex = scrh.tile([P, 512], DT_H, name="lnt")
                    nc.scalar.activation(ex[:, i0:], sc[:, i0:], FX.Exp)
                    nc.vector.tensor_mul(
                        etr[:, jt, i0:], ex[:, i0:], et_sb[:, jt, i0:])
                return etr

            def attn_av(b, hd, etr):
                ot = pot.tile([P, 512], DT_F, name="ot")
                for jt in range(SEQT):
                    i0 = jt * P
                    nc.tensor.matmul(
                        ot[0:65, i0:],
                        vt_b[b][:, jt, hd, :],
                        etr[:, jt, i0:],
                        start=(jt == 0), stop=(jt == SEQT - 1))
                oden = odnp.tile([65, 512], DT_H, name="oden")
                nc.scalar.activation(oden, ot[0:65, :], FX.Identity)
                return oden

            def attn_tail(b, hd, oden):
                po = (hd % 2) * 64
                dt = hd // 2
                tsl = bass.ts(b, 512)
                pden = pm.tile([64, 512], DT_F, name="pmt")
                nc.tensor.matmul(pden, ones_bc[64:65, :], oden[64:65, :],
                                 start=True, stop=True)
                adrb = rdnp.tile([64, 512], DT_F, name="adrb")
                nc.vector.reciprocal_approx_fast(out=adrb, in_=pden)
                if po == 0:
                    nc.vector.tensor_mul(
                        of[0:64, dt, tsl], oden[0:64, :], adrb)
                else:
                    otmp = scrh.tile([64, 512], DT_H, name="lnt2")
                    nc.vector.tensor_mul(otmp, oden[0:64, :], adrb)
                    nc.sync.dma_start(out=of[64:128, dt, tsl], in_=otmp)

            def attn_pipeline(b, units):
                """Heads 0..15 of half b, one filler unit per head slot."""
                etr_d = {}
                oden_d = {}
                ui = [0]

                def pump():
                    if ui[0] < len(units):
                        units[ui[0]]()
                        ui[0] += 1

                for hd in range(HEADS):
                    if hd >= 1:
                        oden_d[hd - 1] = attn_av(b, hd - 1, etr_d.pop(hd - 1))
                    etr_d[hd] = attn_scores(b, hd)
                    pump()
                    if hd >= 2:
                        attn_tail(b, hd - 2, oden_d.pop(hd - 2))
                oden_d[15] = attn_av(b, 15, etr_d.pop(15))
                attn_tail(b, 14, oden_d.pop(14))
                while ui[0] < len(units):
                    pump()
                attn_tail(b, 15, oden_d.pop(15))

            out_wt_cache = {}

            def out_pair(b, dp):
                wt = wsm.tile([P, 2, KTILES, P], DT_H, name="wsm_t")
                nc.sync.dma_start(
                    out=wt,
                    in_=par["wot"][l, 2 * dp:2 * dp + 2].rearrange(
                        "c p k m -> p c k m"))
                tsl = bass.ts(b, 512)
                for di in range(2):
                    dt = 2 * dp + di
                    pq = pm.tile([P, 512], DT_F, name="pmt")
                    for kt in range(KTILES):
                        nc.tensor.matmul(
                            pq, wt[:, di, kt, :], of[:, kt, tsl],
                            start=(kt == 0), stop=(kt == 7))
                    if flags["bo_nz"]:
                        nc.scalar.activation(pq, pq, FX.Identity,
                                             bias=bov_t[:, l, dt:dt + 1])
                    nc.vector.tensor_add(x[:, dt, tsl], pq, x[:, dt, tsl])

            # ---- phase B: qkv b0 (+ LNa th1 bcast mid-way) + v(b0) ----
            with nc.named_scope(f"L{l}_qkv0"):
                if l == 0:
                    for d in range(DTILES):
                        norm_a(0, d)
                for cp in range(8):
                    qkv_pair(0, cp)
                    if cp == 3:
                        planes_a1 = ln_bcast(rows_a1)
                        for d in range(DTILES):
                            norm_a(1, d)
                for nh in range(2):
                    for tth in range(2):
                        v_group(0, nh, tth)

            # ---- att0: heads b0, fillers = qkv(b1) c-tiles + v(b1) ----
            qf1 = qkvp.tile([P, DTILES, 512], DT_H, name="qf")
            kf1 = qkvp.tile([P, DTILES, 512], DT_H, name="kf")
            vt41 = qkvp.tile([P, SEQT, HEADS, 65], DT_H, name="vt")
            nc.vector.memset(vt41[:, :, :, 64:65].bitcast(mybir.dt.uint16),
                             0x3F80)
            qf_b[1], kf_b[1], vt_b[1] = qf1, kf1, vt41

            units0 = (
                [lambda c=c: qkv_half(1, c)
                 for c in (0, 1, 2, 3, 4, 5, 8, 9, 10, 11, 12, 13)]
                + [lambda nh=nh, tth=tth: v_group(1, nh, tth)
                   for nh in range(2) for tth in range(2)]
            )
            with nc.named_scope(f"L{l}_att0"):
                attn_pipeline(0, units0)

            # ---- att1: heads b1, fillers = deferred qkv(b1) + out(b0) ----
            units1 = (
                [lambda c=c: qkv_half(1, c) for c in (6, 14, 7, 15)]
                + [lambda dt=dt: out_half(0, dt) for dt in range(DTILES)]
            )
            with nc.named_scope(f"L{l}_att1"):
                attn_pipeline(1, units1)

            # ---- out tail: LNf stats + out b1 ----
            with nc.named_scope(f"L{l}_out1"):
                psf0 = ln_stats(0)
                rows_f0 = ln_tail(*psf0)
                for dp in range(2):
                    out_pair(1, dp)
                planes_f0 = ln_bcast(rows_f0)
                for dp in range(2, 4):
                    out_pair(1, dp)
                psf1 = ln_stats(1)
                rows_f1 = ln_tail(*psf1)

            h2 = h2p.tile([P, DTILES, 512], DT_H, name="h2")
            h1r = h1p.tile([P, MTILES, 512], DT_H, name="h1r")

            def norm_f(th, d, planes):
                ln_norm(th, d, planes, h2[:, d, :],
                        lnfg_t[:, l, d:d + 1], lnfb_t[:, l, d:d + 1], gb_f)

            def w1_pass(th):
                for mp in range(MTILES // 2):
                    wt = wsm.tile([P, 2, KTILES, P], DT_H, name="wsm_t")
                    nc.sync.dma_start(
                        out=wt,
                        in_=par["w1t"][l, 2 * mp:2 * mp + 2].rearrange(
                            "c p k m -> p c k m"))
                    for mi in range(2):
                        mt = 2 * mp + mi
                        pq = pm.tile([P, 512], DT_F, name="pmt")
                        for kt in range(KTILES):
                            nc.tensor.matmul(
                                pq, wt[:, mi, kt, :], h2[:, kt, :],
                                start=(kt == 0), stop=(kt == 7))
                        nc.scalar.activation(
                            h1r[:, mt, :], pq, FX.Gelu,
                            bias=b1v_t[:, l, mt:mt + 1], scale=1.0)

            def w2_pass(th):
                tsl = bass.ts(th, 512)
                for dt in range(DTILES):
                    pq = pm.tile([P, 512], DT_F, name="pmt")
                    for kh in range(2):
                        wt = wsm.tile([P, 16, P], DT_H, name="wsm_t")
                        nc.sync.dma_start(out=wt, in_=par["w2t"][l, dt, kh])
                        for k2 in range(16):
                            kt = kh * 16 + k2
                            nc.tensor.matmul(
                                pq, wt[:, k2, :], h1r[:, kt, :],
                                start=(kt == 0), stop=(kt == 31))
                    if flags["b2_nz"]:
                        nc.scalar.activation(pq, pq, FX.Identity,
                                             bias=b2v_t[:, l, dt:dt + 1])
                    nc.vector.tensor_add(x[:, dt, tsl], pq, x[:, dt, tsl])

            with nc.named_scope(f"L{l}_ffn"):
                for d in range(DTILES):
                    norm_f(0, d, planes_f0)
                w1_pass(0)
                pf1 = ln_bcast(rows_f1)
                for d in range(DTILES):
                    norm_f(1, d, pf1)
                w2_pass(0)
                if l < DEPTH - 1:
                    psn0 = ln_stats(0)
                    rows_n0 = ln_tail(*psn0)
                w1_pass(1)
                if l < DEPTH - 1:
                    pn0 = ln_bcast(rows_n0)
                    for d in range(DTILES):
                        norm_a(0, d, planes=pn0, lidx=l + 1)
                w2_pass(1)
                if l < DEPTH - 1:
                    psn1 = ln_stats(1)
                    carry_rows_a1 = ln_tail(*psn1)

        # =================== head ===================
        for th in range(2):
            ps = ln_stats(th)
            rows_o = ln_tail(*ps)
            pl_o = ln_bcast(rows_o)
            for d in range(DTILES):
                ln_norm(th, d, pl_o, h[:, d, bass.ts(th, 512)],
                        lnog_t[:, d:d + 1], lnob_t[:, d:d + 1],
                        flags["gb_o"])
        wp3 = par["wproj"].rearrange("(ko p) m -> p ko m", p=P)
        wt = wsm.tile([P, KTILES, NCLS], DT_H, name="wsm_t")
        nc.sync.dma_start(out=wt, in_=wp3)
        out_sb = h2p.tile([NCLS, TOK], DT_F, name="h2")
        for th in range(2):
            pq = pot.tile([P, 512], DT_F, name="ot")
            for kt in range(KTILES):
                nc.tensor.matmul(pq[0:NCLS, :], wt[:, kt, :],
                                 h[:, kt, bass.ts(th, 512)],
                                 start=(kt == 0), stop=(kt == 7))
            nc.scalar.activation(out_sb[:, bass.ts(th, 512)],
                                 pq[0:NCLS, :],
                                 FX.Identity, bias=bproj_t)
        nc.sync.dma_start(out=par["out"][:, :], in_=out_sb)


# ============================================================
# host side
# ============================================================

_NC_CACHE = None
_NC_FLAGS = None


def _bf16(a):
    import ml_dtypes
    return np.ascontiguousarray(a.astype(ml_dtypes.bfloat16))


def _pack_qk(w):      # [D, DIM, 3072] -> [D, 16, P, 8, P]
    v = w[:, :, :2048].reshape(DEPTH, 8, P, 16, P).transpose(0, 3, 2, 1, 4)
    return _bf16(v)


def _pack_v(w):       # -> [D, 2, 8, P, 512]
    v = w[:, :, 2048:].reshape(DEPTH, 8, P, 2, 512).transpose(0, 3, 1, 2, 4)
    return _bf16(v)


def _pack_kxm(w):     # [D, K, M] -> [D, M//P, P, K//P, P]
    D, K, M = w.shape
    v = w.reshape(D, K // P, P, M // P, P).transpose(0, 3, 2, 1, 4)
    return _bf16(v)


def _pack_w2(w):      # [D, 4096, 1024] -> [D, 8, 2, P, 16, P]
    v = w.reshape(DEPTH, 2, 16, P, 8, P).transpose(0, 4, 1, 3, 2, 5)
    return _bf16(v)


def _host_band():
    tt = np.arange(KSIZE, dtype=np.float64)
    kern = np.exp(-0.5 * ((tt - (KSIZE - 1) / 2.0) / SIGMA) ** 2)
    kern = (kern / kern.sum()).astype(np.float32)
    pad_l = (KSIZE - 1) // 2  # 9
    nt = T // P
    bandc = np.zeros((nt, 3, P, P), dtype=np.float32)
    for ct in range(nt):
        for s in range(3):
            kt = ct - 1 + s
            if not (0 <= kt < nt):
                continue
            rows = np.arange(kt * P, (kt + 1) * P)
            cols = np.arange(ct * P, (ct + 1) * P)
            d = rows[:, None] - cols[None, :] + pad_l
            m = (d >= 0) & (d < KSIZE)
            blk = np.zeros((P, P), np.float32)
            blk[m] = kern[d[m]]
            bandc[ct, s] = blk
    return bandc


def _host_etab(rel_tab):
    i = np.arange(SEQ)
    j = i[:, None]
    rel = np.clip(i[None, :] - j, -(MAXREL - 1), MAXREL - 1) + MAXREL - 1
    et = np.zeros((DEPTH, SEQ, SEQ), dtype=np.float32)
    for l in range(DEPTH):
        e = np.exp(rel_tab[l][rel])
        e[j > i[None, :]] = 0.0
        et[l] = e
    return _bf16(et.reshape(DEPTH, SEQT, P, SEQ))


def kernel(**inputs):
    global _NC_CACHE, _NC_FLAGS

    f32 = lambda a: np.ascontiguousarray(np.asarray(a, dtype=np.float32))
    z = lambda a: bool(np.any(np.asarray(a) != 0))
    one = lambda a: bool(np.all(np.asarray(a) == 1.0))
    flags = {
        "gb_a": (not one(inputs["ln_a_g"])) or z(inputs["ln_a_b"]),
        "gb_f": (not one(inputs["ln_f_g"])) or z(inputs["ln_f_b"]),
        "gb_o": (not one(inputs["ln_o_g"])) or z(inputs["ln_o_b"]),
        "bo_nz": z(inputs["bo"]),
        "b2_nz": z(inputs["b2"]),
    }
    if _NC_CACHE is None or _NC_FLAGS != flags:
        _NC_CACHE = build_nc(flags)
        _NC_FLAGS = dict(flags)
    nc = _NC_CACHE

    def vx(a):       # [width] -> [P, width//P]  (partition-major)
        a = f32(a)
        return np.ascontiguousarray(a.reshape(-1, P).T)

    def vxl(a):      # [L, width] -> [P, L, width//P]
        a = f32(a)
        L = a.shape[0]
        return np.ascontiguousarray(a.reshape(L, -1, P).transpose(2, 0, 1))

    shared = {
        "band": _host_band(),
        "etab": _host_etab(f32(inputs["rel_tab"])),
        "wpe": f32(inputs["W_pe"]),
        "wqk_t": _pack_qk(f32(inputs["Wqkv"])),
        "wv_t": _pack_v(f32(inputs["Wqkv"])),
        "wot": _pack_kxm(f32(inputs["Wo"])),
        "w1t": _pack_kxm(f32(inputs["W1"])),
        "w2t": _pack_w2(f32(inputs["W2"])),
        "wproj": _bf16(f32(inputs["Wproj"])),
        "lnp1g": vx(inputs["ln_p1_g"]), "lnp1b": vx(inputs["ln_p1_b"]),
        "bpe": vx(inputs["b_pe"]),
        "lnp2g": vx(inputs["ln_p2_g"]), "lnp2b": vx(inputs["ln_p2_b"]),
        "lnag": vxl(inputs["ln_a_g"]), "lnab": vxl(inputs["ln_a_b"]),
        "lnfg": vxl(inputs["ln_f_g"]), "lnfb": vxl(inputs["ln_f_b"]),
        "bov": vxl(inputs["bo"]), "b1v": vxl(inputs["b1"]),
        "b2v": vxl(inputs["b2"]),
        "lnog": vx(inputs["ln_o_g"]), "lnob": vx(inputs["ln_o_b"]),
        "bprojv": f32(inputs["bproj"]),
    }
    xfull = f32(inputs["neuralInput"])
    in_maps = []
    for c in range(NCORES):
        m = dict(shared)
        m["xin"] = np.ascontiguousarray(xfull[c * BPC:(c + 1) * BPC])
        in_maps.append(m)

    import os
    trace = bool(os.environ.get("BIT_TRACE"))
    res = run_bass_kernel_spmd(nc, in_maps, list(range(NCORES)), trace=trace)
    if trace:
        globals()["LAST_RESULT"] = res
    outs = []
    for c in range(NCORES):
        o = res.results[c]["out"]              # [NCLS, TOK]
        o = o.reshape(NCLS, BPC, SEQ).transpose(1, 2, 0)
        outs.append(o)
    return np.concatenate(outs, axis=0).astype(np.float32)


# revision 34
# speedup vs baseline: 1.2343x; 1.0065x over previous
"""Trainium2 Bass kernel for nn_BiT_Phoneme (dense transformer), v3.

Data-parallel: 16 batch elems / 8 cores = 2 per core. Feature-major
activations ([dim on partitions, tokens on free]); residual x in bf16.
v3 keeps the PE warm through attention (HAM was throttling ~75% of the
attention region in v2):
  - attention heads run as a 3-stage software pipeline per half:
    scores(h) / attnV(h-1) / den-tail(h-2), with one dense filler unit
    (qkv c-tile, v group, or out-proj dtile) per head slot so the PE
    never idles long enough to re-throttle.
  - den tail: reciprocal on the [1,512] den row, then K=1 broadcast
    matmul; of-mul reads the broadcast straight from PSUM (drops the
    dsb copy + [64,512] reciprocal of v2).
  - oden evac moved to ACT identity (DVE was the attention bottleneck).
  - LN stats use DVE add-trees + 2 matmuls instead of 16 stat matmuls.
  - out-proj weights double-pumped: b0 as att1 fillers, b1 after.
"""

import numpy as np

import concourse.bass as bass
import concourse.mybir as mybir
import concourse.tile as tile
from concourse import bacc
from concourse.bass_utils import run_bass_kernel_spmd

B, T, F = 16, 2048, 256
PH = 4
PATCH = 1024
DIM = 1024
DEPTH = 6
HEADS, DHEAD = 16, 64
INNER = 1024
MLP = 4096
NCLS = 41
MAXREL = 200
KSIZE, SIGMA = 20, 2.0
EPS = 1e-5
SEQ = T // PH              # 512
NCORES = 8
BPC = B // NCORES          # 2
TOK = BPC * SEQ            # 1024
P = 128

DT_R = mybir.dt.float32r
DT_F = mybir.dt.float32
DT_H = mybir.dt.bfloat16
FX = mybir.ActivationFunctionType
OP = mybir.AluOpType

DTILES = DIM // P          # 8
KTILES = DIM // P          # 8
MTILES = MLP // P          # 32
SEQT = SEQ // P            # 4


def build_nc(flags):
    nc = bacc.Bacc(None, target_bir_lowering=False)

    par = {}
    def dp(name, shape, dtype, is_out=False):
        par[name] = nc.declare_dram_parameter(name, list(shape), dtype, isOutput=is_out)
        return par[name]

    dp("xin", (BPC, T, F), DT_R)
    dp("band", (T // P, 3, P, P), DT_R)
    dp("etab", (DEPTH, SEQT, P, SEQ), DT_H)
    dp("wpe", (PATCH, DIM), DT_R)
    dp("wqk_t", (DEPTH, 16, P, KTILES, P), DT_H)
    dp("wv_t", (DEPTH, 2, KTILES, P, 512), DT_H)
    dp("wot", (DEPTH, DTILES, P, KTILES, P), DT_H)
    dp("w1t", (DEPTH, MTILES, P, KTILES, P), DT_H)
    dp("w2t", (DEPTH, DTILES, 2, P, 16, P), DT_H)
    dp("wproj", (DIM, NCLS), DT_H)
    # vectors arrive pre-transposed to [P, (L,) width//P] so the DMA is
    # one contiguous run per partition instead of an element gather
    for nm, width, L in [("lnp1g", PATCH, 0), ("lnp1b", PATCH, 0),
                         ("bpe", DIM, 0), ("lnp2g", DIM, 0),
                         ("lnp2b", DIM, 0),
                         ("lnag", DIM, DEPTH), ("lnab", DIM, DEPTH),
                         ("lnfg", DIM, DEPTH), ("lnfb", DIM, DEPTH),
                         ("bov", DIM, DEPTH), ("b1v", MLP, DEPTH),
                         ("b2v", DIM, DEPTH), ("lnog", DIM, 0),
                         ("lnob", DIM, 0)]:
        shp = (P, L, width // P) if L else (P, width // P)
        dp(nm, shp, DT_F)
    dp("bprojv", (NCLS,), DT_F)
    dp("out", (NCLS, TOK), DT_F, is_out=True)

    with tile.TileContext(nc) as tc:
        _emit(nc, tc, par, flags)
    nc.compile()
    return nc


def _emit(nc, tc, par, flags):
    import contextlib
    ctx = contextlib.ExitStack()
    with ctx:
        const = ctx.enter_context(tc.tile_pool(name="const", bufs=1))
        xpool = ctx.enter_context(tc.tile_pool(name="xpool", bufs=1))
        wsm = ctx.enter_context(tc.tile_pool(name="wsm", bufs=5))
        scr = ctx.enter_context(tc.tile_pool(name="scr", bufs=4))
        scrh = ctx.enter_context(tc.tile_pool(name="scrh", bufs=2))
        stp = ctx.enter_context(tc.tile_pool(name="stp", bufs=2))
        plh = ctx.enter_context(tc.tile_pool(name="plh", bufs=4))
        rowp = ctx.enter_context(tc.tile_pool(name="rowp", bufs=2))
        odnp = ctx.enter_context(tc.tile_pool(name="odnp", bufs=2))
        rdnp = ctx.enter_context(tc.tile_pool(name="rdnp", bufs=2))
        pm = ctx.enter_context(tc.tile_pool(name="pm", bufs=4, space="PSUM"))
        pot = ctx.enter_context(tc.tile_pool(name="pot", bufs=2, space="PSUM"))
        pst = ctx.enter_context(tc.tile_pool(name="pst", bufs=1, space="PSUM"))

        ones_r = const.tile([P, 1], DT_R, name="ones_r")
        nc.vector.memset(ones_r.bitcast(mybir.dt.uint32), 0x3F800000)
        ones_h = const.tile([P, 1], DT_H, name="ones_h")
        nc.vector.memset(ones_h.bitcast(mybir.dt.uint16), 0x3F80)
        ones_row = const.tile([1, P], DT_R, name="ones_row")
        nc.vector.memset(ones_row.bitcast(mybir.dt.uint32), 0x3F800000)
        ones_bc = const.tile([P, 64], DT_H, name="ones_bc")
        nc.vector.memset(ones_bc.bitcast(mybir.dt.uint16), 0x3F80)
        epst = const.tile([1, 1], DT_F, name="epst")
        nc.vector.memset(epst, EPS)

        def load_vec(nm, width):
            d = par[nm]
            tl = const.tile(list(d.shape), DT_F, name=nm + "_t")
            nc.sync.dma_start(out=tl, in_=d[tuple(slice(None) for _ in d.shape)])
            return tl

        x = xpool.tile([P, DTILES, TOK], DT_H, name="x")

        # ---------- embedding layernorms (split stats / tail / finish) ----
        def emb_stats(views, use_ot):
            """16 accumulating stat matmuls over 8 f32r [P,512] views."""
            if use_ot:
                ps0 = pot.tile([1, 512], DT_F, name="ot")
                ps1 = pot.tile([1, 512], DT_F, name="ot")
            else:
                ps0 = pst.tile([1, 512], DT_F, name="ps0")
                ps1 = pst.tile([1, 512], DT_F, name="ps1")
            for d in range(8):
                v = views(d)
                sq = scr.tile([P, 512], DT_R, name="scr")
                nc.vector.tensor_mul(sq, v, v)
                nc.tensor.matmul(ps0, ones_r, v, start=(d == 0), stop=(d == 7))
                nc.tensor.matmul(ps1, ones_r, sq, start=(d == 0), stop=(d == 7))
            return ps0, ps1

        def emb_finish(views, dst_fn, rows, g_fn, b_fn, apply_gb):
            """bcast (r, mu*r) planes and normalize 8 views into dst."""
            p1, p2 = ln_bcast(rows)
            for d in range(8):
                t = scr.tile([P, 512], DT_F, name="scr")
                nc.vector.tensor_mul(t, views(d), p1)
                if apply_gb:
                    nc.vector.tensor_sub(t, t, p2)
                    nc.vector.tensor_scalar(dst_fn(d), t, g_fn(d), b_fn(d),
                                            OP.mult, OP.add)
                else:
                    nc.vector.tensor_sub(dst_fn(d), t, p2)

        def ln_tail(ps0, ps1):
            rows = rowp.tile([1, 2, 512], DT_R, name="rows")
            a = scr.tile([1, 512], DT_F, name="scr")
            v = scr.tile([1, 512], DT_F, name="scr")
            m2 = scr.tile([1, 512], DT_F, name="scr")
            nc.vector.tensor_scalar(a, ps0, 1.0 / DIM, None, OP.mult)
            nc.vector.tensor_scalar(v, ps1, 1.0 / DIM, None, OP.mult)
            nc.vector.tensor_mul(m2, a, a)
            nc.vector.tensor_sub(v, v, m2)
            nc.scalar.activation(v, v, FX.Sqrt, bias=epst, scale=1.0)
            rv = scr.tile([1, 512], DT_F, name="scr")
            nc.vector.reciprocal_approx_fast(out=rv, in_=v)
            nc.vector.tensor_copy(rows[:, 0, :], rv)
            nc.vector.tensor_mul(rows[:, 1, :], a, rv)
            return rows

        def ln_bcast(rows):
            out = []
            for s in range(2):
                pp = pm.tile([P, 512], DT_F, name="pmt")
                nc.tensor.matmul(pp, ones_row, rows[:, s, :],
                                 start=True, stop=True)
                psb = plh.tile([P, 512], DT_H, name="plh")
                nc.scalar.activation(psb, pp, FX.Identity)
                out.append(psb)
            return tuple(out)

        # =================== embedding ===================
        xin, band = par["xin"], par["band"]
        with (
            tc.tile_pool(name="sfp", bufs=1) as sfp,
            tc.tile_pool(name="pnp", bufs=1) as pnp,
            tc.tile_pool(name="x0p", bufs=1) as x0p,
            tc.tile_pool(name="xap", bufs=16) as xap,
        ):
            x0 = x0p.tile([P, DTILES, TOK], DT_R, name="x0")
            sf4 = sfp.tile([P, BPC, 2, T], DT_R, name="sf4")
            bfh = [(b, fh) for b in range(BPC) for fh in range(2)]
            xa_cache = {}

            def get_xa(b, kt, fh):
                key = (b, kt, fh)
                if key not in xa_cache:
                    t = xap.tile([P, P], DT_R, name="xa")
                    nc.sync.dma_start(
                        out=t, in_=xin[b, bass.ts(kt, P), bass.ts(fh, P)])
                    xa_cache[key] = t
                return xa_cache[key]

            # prefetch the first few xa tiles before anything else queues
            for kt in range(2):
                for (b, fh) in bfh:
                    get_xa(b, kt, fh)

            lnp1g_t = load_vec("lnp1g", PATCH)
            lnp1b_t = load_vec("lnp1b", PATCH)
            bpe_t = load_vec("bpe", DIM)
            lnp2g_t = load_vec("lnp2g", DIM)
            lnp2b_t = load_vec("lnp2b", DIM)
            lnag_t = load_vec("lnag", DIM)
            lnab_t = load_vec("lnab", DIM)
            lnfg_t = load_vec("lnfg", DIM)
            lnfb_t = load_vec("lnfb", DIM)
            bov_t = load_vec("bov", DIM)
            b1v_t = load_vec("b1v", MLP)
            b2v_t = load_vec("b2v", DIM)
            lnog_t = load_vec("lnog", DIM)
            lnob_t = load_vec("lnob", DIM)
            bproj_t = const.tile([NCLS, 1], DT_F, name="bproj_t")
            nc.sync.dma_start(out=bproj_t,
                              in_=par["bprojv"].rearrange("(p o) -> p o", o=1))

            for g4 in range(T // 512):
                pgs = {}
                for i, (b, fh) in enumerate(bfh):
                    pool = pm if i < 2 else pot
                    pgs[(b, fh)] = pool.tile([P, 512], DT_F,
                                             name="pmt" if i < 2 else "ot")
                for q in range(4):
                    ct = g4 * 4 + q
                    bt = wsm.tile([P, 3, P], DT_R, name="wsm_t")
                    nc.sync.dma_start(
                        out=bt, in_=band[ct].rearrange("s p q -> p s q"))
                    svals = [s for s in range(3)
                             if 0 <= ct - 1 + s < T // P]
                    for (b, fh) in bfh:
                        for si, s in enumerate(svals):
                            kt = ct - 1 + s
                            xa = get_xa(b, kt, fh)
                            nc.tensor.matmul(
                                pgs[(b, fh)][:, bass.ts(q, P)],
                                xa,
                                bt[:, s, :],
                                start=(q == 0 and si == 0),
                                stop=(q == 3 and si == len(svals) - 1))
                for (b, fh) in bfh:
                    nc.vector.tensor_copy(
                        sf4[:, b, fh, bass.ts(g4, 512)], pgs[(b, fh)])

            pn = pnp.tile([P, BPC, 8, 512], DT_R, name="pn")

            def mk_pview(b):
                def pview(pt):
                    i, fh = pt // 2, pt % 2
                    return sf4[:, b, fh, :].rearrange(
                        "p (s four) -> p four s", four=PH)[:, i, :]
                return pview

            pviews = [mk_pview(b) for b in range(BPC)]
            # b1 stats cover b0's tail chain; finishes pipeline likewise
            s0 = emb_stats(pviews[0], False)
            s1 = emb_stats(pviews[1], True)
            r0 = ln_tail(*s0)
            r1 = ln_tail(*s1)
            emb_finish(pviews[0], lambda d: pn[:, 0, d, :], r0,
                       lambda d: lnp1g_t[:, d:d + 1],
                       lambda d: lnp1b_t[:, d:d + 1], flags["gb_p1"])
            emb_finish(pviews[1], lambda d: pn[:, 1, d, :], r1,
                       lambda d: lnp1g_t[:, d:d + 1],
                       lambda d: lnp1b_t[:, d:d + 1], flags["gb_p1"])

            # wpe matmuls with x0-LN stats folded into the stream
            psxS = [pst.tile([1, 512], DT_F, name="ps0"),
                    pot.tile([1, 512], DT_F, name="ot")]
            psxQ = [pst.tile([1, 512], DT_F, name="ps1"),
                    pot.tile([1, 512], DT_F, name="ot")]
            for dt in range(DTILES):
                wt = wsm.tile([P, KTILES, P], DT_R, name="wsm_t")
                nc.sync.dma_start(
                    out=wt,
                    in_=par["wpe"].rearrange("(ko p) m -> p ko m", p=P)[
                        :, :, bass.ts(dt, P)])
                for b in range(BPC):
                    pq = pm.tile([P, 512], DT_F, name="pmt")
                    for kt in range(KTILES):
                        nc.tensor.matmul(pq, wt[:, kt, :], pn[:, b, kt, :],
                                         start=(kt == 0), stop=(kt == 7))
                    v = x0[:, dt, bass.ts(b, 512)]
                    nc.vector.tensor_scalar(
                        v, pq, bpe_t[:, dt:dt + 1], None, OP.add)
                    sq = scr.tile([P, 512], DT_R, name="scr")
                    nc.vector.tensor_mul(sq, v, v)
                    nc.tensor.matmul(psxS[b], ones_r, v,
                                     start=(dt == 0), stop=(dt == DTILES - 1))
                    nc.tensor.matmul(psxQ[b], ones_r, sq,
                                     start=(dt == 0), stop=(dt == DTILES - 1))
            rows_x = [ln_tail(psxS[th], psxQ[th]) for th in range(2)]
            for th in range(2):
                emb_finish(lambda d, th=th: x0[:, d, bass.ts(th, 512)],
                           lambda d, th=th: x[:, d, bass.ts(th, 512)],
                           rows_x[th],
                           lambda d: lnp2g_t[:, d:d + 1],
                           lambda d: lnp2b_t[:, d:d + 1], flags["gb_p2"])

        # main pools (opened after embedding scratch closes)
        hpool = ctx.enter_context(tc.tile_pool(name="hpool", bufs=1))
        h2p = ctx.enter_context(tc.tile_pool(name="h2p", bufs=1))
        qkvp = ctx.enter_context(tc.tile_pool(name="qkvp", bufs=2))
        ofp = ctx.enter_context(tc.tile_pool(name="ofp", bufs=1))
        h1p = ctx.enter_context(tc.tile_pool(name="h1p", bufs=1))
        etrp = ctx.enter_context(tc.tile_pool(name="etrp", bufs=2))
        etp = ctx.enter_context(tc.tile_pool(name="etp", bufs=1))

        h = hpool.tile([P, DTILES, TOK], DT_H, name="h")
        of = ofp.tile([P, DTILES, TOK], DT_H, name="of")

        # ---------- split layernorm helpers (main layers) ----------
        def ln_stats(th):
            """DVE add-chains for sum/sumsq + 2 partition-reduce matmuls."""
            sl = bass.ts(th, 512)
            ps0 = pst.tile([1, 512], DT_F, name="ps0")
            ps1 = pst.tile([1, 512], DT_F, name="ps1")

            flip = [0]

            def acc_tile():
                t = stp.tile([P, 512], DT_H,
                             name=("ta" if flip[0] else "tb"))
                flip[0] ^= 1
                return t

            sacc = x[:, 0, sl]
            for d in range(1, DTILES):
                t = acc_tile()
                nc.vector.tensor_add(t, sacc, x[:, d, sl])
                sacc = t
            nc.tensor.matmul(ps0, ones_h, sacc, start=True, stop=True)
            qacc = None
            for d in range(DTILES):
                q = stp.tile([P, 512], DT_H, name="sq")
                nc.vector.tensor_mul(q, x[:, d, sl], x[:, d, sl])
                if qacc is None:
                    qacc = q
                else:
                    t = acc_tile()
                    nc.vector.tensor_add(t, qacc, q)
                    qacc = t
            nc.tensor.matmul(ps1, ones_h, qacc, start=True, stop=True)
            return ps0, ps1

        def ln_norm(th, d, planes, dst, g_ap, b_ap, apply_gb):
            sl = bass.ts(th, 512)
            p1, p2 = planes
            t = scrh.tile([P, 512], DT_H, name="lnt")
            nc.vector.tensor_mul(t, x[:, d, sl], p1)
            if apply_gb:
                t2 = scrh.tile([P, 512], DT_H, name="lnt2")
                nc.vector.tensor_sub(t2, t, p2)
                nc.vector.tensor_scalar(dst, t2, g_ap, b_ap, OP.mult, OP.add)
            else:
                nc.vector.tensor_sub(dst, t, p2)

        # =================== transformer layers ===================
        carry_rows_a1 = None
        for l in range(DEPTH):
            gb_a = flags["gb_a"]
            gb_f = flags["gb_f"]

            et_sb = etp.tile([P, SEQT, SEQ], DT_H, name="et_sb")
            nc.sync.dma_start(out=et_sb,
                              in_=par["etab"][l].rearrange("jt p i -> p jt i"))

            if l == 0:
                with nc.named_scope(f"L{l}_lna"):
                    psa0 = ln_stats(0)
                    rows_a0 = ln_tail(*psa0)
                    psa1 = ln_stats(1)
                    rows_a1 = ln_tail(*psa1)
                    planes_a0 = ln_bcast(rows_a0)
            else:
                rows_a1 = carry_rows_a1

            qf = qkvp.tile([P, DTILES, 512], DT_H, name="qf")
            kf = qkvp.tile([P, DTILES, 512], DT_H, name="kf")
            vt4 = qkvp.tile([P, SEQT, HEADS, 65], DT_H, name="vt")
            nc.vector.memset(vt4[:, :, :, 64:65].bitcast(mybir.dt.uint16),
                             0x3F80)
            qf_b = [qf, None]
            kf_b = [kf, None]
            vt_b = [vt4, None]

            def norm_a(th, d, planes=None, lidx=None):
                li = l if lidx is None else lidx
                pl_ = planes if planes is not None else (
                    planes_a0 if th == 0 else planes_a1)
                ln_norm(th, d, pl_, h[:, d, bass.ts(th, 512)],
                        lnag_t[:, li, d:d + 1], lnab_t[:, li, d:d + 1], gb_a)

            qkv_wt_cache = {}

            def qkv_half(b, c):
                cp, ci = c // 2, c % 2
                if ci == 0:
                    wt = wsm.tile([P, 2, KTILES, P], DT_H, name="wsm_t")
                    nc.sync.dma_start(
                        out=wt,
                        in_=par["wqk_t"][l, 2 * cp:2 * cp + 2].rearrange(
                            "c p k m -> p c k m"))
                    qkv_wt_cache[(b, cp)] = wt
                wt = qkv_wt_cache[(b, cp)]
                tsl = bass.ts(b, 512)
                pq = pm.tile([P, 512], DT_F, name="pmt")
                for kt in range(KTILES):
                    nc.tensor.matmul(pq, wt[:, ci, kt, :], h[:, kt, tsl],
                                     start=(kt == 0), stop=(kt == 7))
                if c < DTILES:
                    nc.vector.tensor_copy(qf_b[b][:, c, :], pq)
                else:
                    nc.vector.tensor_scalar(
                        kf_b[b][:, c - DTILES, :], pq,
                        float(DHEAD) ** -0.5, None, OP.mult)

            def out_half(b, dt):
                dp, di = dt // 2, dt % 2
                if di == 0:
                    wt = wsm.tile([P, 2, KTILES, P], DT_H, name="wsm_t")
                    nc.sync.dma_start(
                        out=wt,
                        in_=par["wot"][l, 2 * dp:2 * dp + 2].rearrange(
                            "c p k m -> p c k m"))
                    out_wt_cache[(b, dp)] = wt
                wt = out_wt_cache[(b, dp)]
                tsl = bass.ts(b, 512)
                pq = pm.tile([P, 512], DT_F, name="pmt")
                for kt in range(KTILES):
                    nc.tensor.matmul(
                        pq, wt[:, di, kt, :], of[:, kt, tsl],
                        start=(kt == 0), stop=(kt == 7))
                if flags["bo_nz"]:
                    nc.scalar.activation(pq, pq, FX.Identity,
                                         bias=bov_t[:, l, dt:dt + 1])
                nc.vector.tensor_add(x[:, dt, tsl], pq, x[:, dt, tsl])

            def qkv_pair(b, cp):
                wt = wsm.tile([P, 2, KTILES, P], DT_H, name="wsm_t")
                nc.sync.dma_start(
                    out=wt,
                    in_=par["wqk_t"][l, 2 * cp:2 * cp + 2].rearrange(
                        "c p k m -> p c k m"))
                tsl = bass.ts(b, 512)
                for ci in range(2):
                    c = 2 * cp + ci
                    pq = pm.tile([P, 512], DT_F, name="pmt")
                    for kt in range(KTILES):
                        nc.tensor.matmul(pq, wt[:, ci, kt, :], h[:, kt, tsl],
                                         start=(kt == 0), stop=(kt == 7))
                    if c < DTILES:
                        nc.scalar.activation(qf_b[b][:, c, :], pq, FX.Identity)
                    else:
                        nc.scalar.activation(
                            kf_b[b][:, c - DTILES, :], pq, FX.Identity,
                            scale=float(DHEAD) ** -0.5)

            def v_group(b, nh, tth):
                tts = (2 * tth, 2 * tth + 1)
                pvs = [pm.tile([P, 512], DT_F, name="pmt") for _ in tts]
                for kp in range(KTILES // 2):
                    wv = wsm.tile([P, 2, 512], DT_H, name="wsm_t")
                    nc.sync.dma_start(
                        out=wv,
                        in_=par["wv_t"][l, nh, 2 * kp:2 * kp + 2].rearrange(
                            "k p n -> p k n"))
                    for ki in range(2):
                        kt = 2 * kp + ki
                        for ti, tt in enumerate(tts):
                            nc.tensor.matmul(
                                pvs[ti],
                                h[:, kt, b * 512 + tt * P:
                                  b * 512 + (tt + 1) * P],
                                wv[:, ki, :],
                                start=(kt == 0), stop=(kt == 7))
                for ti, tt in enumerate(tts):
                    nc.scalar.activation(
                        vt_b[b][:, tt, nh * 8:(nh + 1) * 8, 0:64],
                        pvs[ti].rearrange("p (hd d) -> p hd d", d=64),
                        FX.Identity)

            # ---- pipelined attention stages ----
            def attn_scores(b, hd):
                po = (hd % 2) * 64
                dt = hd // 2
                etr = etrp.tile([P, SEQT, 512], DT_H, name="etr")
                for jt in range(SEQT):
                    i0 = jt * P
                    sc = pm.tile([P, 512], DT_F, name="pmt")
                    nc.tensor.matmul(
                        sc[:, i0:],
                        kf_b[b][po:po + 64, dt, bass.ts(jt, P)],
                        qf_b[b][po:po + 64, dt, i0:],
                        start=True, stop=True)
                    ex = scrh.tile([P, 512], DT_H, name="lnt")
                    nc.scalar.activation(ex[:, i0:], sc[:, i0:], FX.Exp)
                    nc.vector.tensor_mul(
                        etr[:, jt, i0:], ex[:, i0:], et_sb[:, jt, i0:])
                return etr

            def attn_av(b, hd, etr):
                ot = pot.tile([P, 512], DT_F, name="ot")
                for jt in range(SEQT):
                    i0 = jt * P
                    nc.tensor.matmul(
                        ot[0:65, i0:],
                        vt_b[b][:, jt, hd, :],
                        etr[:, jt, i0:],
                        start=(jt == 0), stop=(jt == SEQT - 1))
                oden = odnp.tile([65, 512], DT_H, name="oden")
                nc.scalar.activation(oden, ot[0:65, :], FX.Identity)
                return oden

            def attn_tail(b, hd, oden):
                po = (hd % 2) * 64
                dt = hd // 2
                tsl = bass.ts(b, 512)
                pden = pm.tile([64, 512], DT_F, name="pmt")
                nc.tensor.matmul(pden, ones_bc[64:65, :], oden[64:65, :],
                                 start=True, stop=True)
                adrb = rdnp.tile([64, 512], DT_F, name="adrb")
                nc.vector.reciprocal_approx_fast(out=adrb, in_=pden)
                if po == 0:
                    nc.vector.tensor_mul(
                        of[0:64, dt, tsl], oden[0:64, :], adrb)
                else:
                    otmp = scrh.tile([64, 512], DT_H, name="lnt2")
                    nc.vector.tensor_mul(otmp, oden[0:64, :], adrb)
                    nc.sync.dma_start(out=of[64:128, dt, tsl], in_=otmp)

            def attn_pipeline(b, units):
                """Heads 0..15 of half b, one filler unit per head slot."""
                etr_d = {}
                oden_d = {}
                ui = [0]

                def pump():
                    if ui[0] < len(units):
                        units[ui[0]]()
                        ui[0] += 1

                for hd in range(HEADS):
                    if hd >= 1:
                        oden_d[hd - 1] = attn_av(b, hd - 1, etr_d.pop(hd - 1))
                    etr_d[hd] = attn_scores(b, hd)
                    pump()
                    if hd >= 2:
                        attn_tail(b, hd - 2, oden_d.pop(hd - 2))
                oden_d[15] = attn_av(b, 15, etr_d.pop(15))
                attn_tail(b, 14, oden_d.pop(14))
                while ui[0] < len(units):
                    pump()
                attn_tail(b, 15, oden_d.pop(15))

            out_wt_cache = {}

            def out_pair(b, dp):
                wt = wsm.tile([P, 2, KTILES, P], DT_H, name="wsm_t")
                nc.sync.dma_start(
                    out=wt,
                    in_=par["wot"][l, 2 * dp:2 * dp + 2].rearrange(
                        "c p k m -> p c k m"))
                tsl = bass.ts(b, 512)
                for di in range(2):
                    dt = 2 * dp + di
                    pq = pm.tile([P, 512], DT_F, name="pmt")
                    for kt in range(KTILES):
                        nc.tensor.matmul(
                            pq, wt[:, di, kt, :], of[:, kt, tsl],
                            start=(kt == 0), stop=(kt == 7))
                    if flags["bo_nz"]:
                        nc.scalar.activation(pq, pq, FX.Identity,
                                             bias=bov_t[:, l, dt:dt + 1])
                    nc.vector.tensor_add(x[:, dt, tsl], pq, x[:, dt, tsl])

            # ---- phase B: qkv b0 (+ LNa th1 bcast mid-way) + v(b0) ----
            with nc.named_scope(f"L{l}_qkv0"):
                if l == 0:
                    for d in range(DTILES):
                        norm_a(0, d)
                for cp in range(8):
                    qkv_pair(0, cp)
                    if cp == 3:
                        planes_a1 = ln_bcast(rows_a1)
                        for d in range(DTILES):
                            norm_a(1, d)
                for nh in range(2):
                    for tth in range(2):
                        v_group(0, nh, tth)

            # ---- att0: heads b0, fillers = qkv(b1) c-tiles + v(b1) ----
            qf1 = qkvp.tile([P, DTILES, 512], DT_H, name="qf")
            kf1 = qkvp.tile([P, DTILES, 512], DT_H, name="kf")
            vt41 = qkvp.tile([P, SEQT, HEADS, 65], DT_H, name="vt")
            nc.vector.memset(vt41[:, :, :, 64:65].bitcast(mybir.dt.uint16),
                             0x3F80)
            qf_b[1], kf_b[1], vt_b[1] = qf1, kf1, vt41

            units0 = (
                [lambda c=c: qkv_half(1, c)
                 for c in (0, 1, 2, 3, 4, 5, 8, 9, 10, 11, 12, 13)]
                + [lambda nh=nh, tth=tth: v_group(1, nh, tth)
                   for nh in range(2) for tth in range(2)]
            )
            with nc.named_scope(f"L{l}_att0"):
                attn_pipeline(0, units0)

            # ---- att1: heads b1, fillers = deferred qkv(b1) + out(b0) ----
            units1 = (
                [lambda c=c: qkv_half(1, c) for c in (6, 14, 7, 15)]
                + [lambda dt=dt: out_half(0, dt) for dt in range(DTILES)]
            )
            with nc.named_scope(f"L{l}_att1"):
                attn_pipeline(1, units1)

            # ---- out tail: LNf stats + out b1 ----
            with nc.named_scope(f"L{l}_out1"):
                psf0 = ln_stats(0)
                rows_f0 = ln_tail(*psf0)
                for dp in range(2):
                    out_pair(1, dp)
                planes_f0 = ln_bcast(rows_f0)
                for dp in range(2, 4):
                    out_pair(1, dp)
                psf1 = ln_stats(1)
                rows_f1 = ln_tail(*psf1)

            h2 = h2p.tile([P, DTILES, 512], DT_H, name="h2")
            h1r = h1p.tile([P, MTILES, 512], DT_H, name="h1r")

            def norm_f(th, d, planes):
                ln_norm(th, d, planes, h2[:, d, :],
                        lnfg_t[:, l, d:d + 1], lnfb_t[:, l, d:d + 1], gb_f)

            def w1_pass(th):
                for mp in range(MTILES // 2):
                    wt = wsm.tile([P, 2, KTILES, P], DT_H, name="wsm_t")
                    nc.sync.dma_start(
                        out=wt,
                        in_=par["w1t"][l, 2 * mp:2 * mp + 2].rearrange(
                            "c p k m -> p c k m"))
                    for mi in range(2):
                        mt = 2 * mp + mi
                        pq = pm.tile([P, 512], DT_F, name="pmt")
                        for kt in range(KTILES):
                            nc.tensor.matmul(
                                pq, wt[:, mi, kt, :], h2[:, kt, :],
                                start=(kt == 0), stop=(kt == 7))
                        nc.scalar.activation(
                            h1r[:, mt, :], pq, FX.Gelu,
                            bias=b1v_t[:, l, mt:mt + 1], scale=1.0)

            def w2_pass(th):
                tsl = bass.ts(th, 512)
                for dt in range(DTILES):
                    pq = pm.tile([P, 512], DT_F, name="pmt")
                    for kh in range(2):
                        wt = wsm.tile([P, 16, P], DT_H, name="wsm_t")
                        nc.sync.dma_start(out=wt, in_=par["w2t"][l, dt, kh])
                        for k2 in range(16):
                            kt = kh * 16 + k2
                            nc.tensor.matmul(
                                pq, wt[:, k2, :], h1r[:, kt, :],
                                start=(kt == 0), stop=(kt == 31))
                    if flags["b2_nz"]:
                        nc.scalar.activation(pq, pq, FX.Identity,
                                             bias=b2v_t[:, l, dt:dt + 1])
                    nc.vector.tensor_add(x[:, dt, tsl], pq, x[:, dt, tsl])

            with nc.named_scope(f"L{l}_ffn"):
                for d in range(DTILES):
                    norm_f(0, d, planes_f0)
                w1_pass(0)
                pf1 = ln_bcast(rows_f1)
                for d in range(DTILES):
                    norm_f(1, d, pf1)
                w2_pass(0)
                if l < DEPTH - 1:
                    psn0 = ln_stats(0)
                    rows_n0 = ln_tail(*psn0)
                w1_pass(1)
                if l < DEPTH - 1:
                    pn0 = ln_bcast(rows_n0)
                    for d in range(DTILES):
                        norm_a(0, d, planes=pn0, lidx=l + 1)
                w2_pass(1)
                if l < DEPTH - 1:
                    psn1 = ln_stats(1)
                    carry_rows_a1 = ln_tail(*psn1)

        # =================== head ===================
        for th in range(2):
            ps = ln_stats(th)
            rows_o = ln_tail(*ps)
            pl_o = ln_bcast(rows_o)
            for d in range(DTILES):
                ln_norm(th, d, pl_o, h[:, d, bass.ts(th, 512)],
                        lnog_t[:, d:d + 1], lnob_t[:, d:d + 1],
                        flags["gb_o"])
        wp3 = par["wproj"].rearrange("(ko p) m -> p ko m", p=P)
        wt = wsm.tile([P, KTILES, NCLS], DT_H, name="wsm_t")
        nc.sync.dma_start(out=wt, in_=wp3)
        out_sb = h2p.tile([NCLS, TOK], DT_F, name="h2")
        for th in range(2):
            pq = pot.tile([P, 512], DT_F, name="ot")
            for kt in range(KTILES):
                nc.tensor.matmul(pq[0:NCLS, :], wt[:, kt, :],
                                 h[:, kt, bass.ts(th, 512)],
                                 start=(kt == 0), stop=(kt == 7))
            nc.scalar.activation(out_sb[:, bass.ts(th, 512)],
                                 pq[0:NCLS, :],
                                 FX.Identity, bias=bproj_t)
        nc.sync.dma_start(out=par["out"][:, :], in_=out_sb)


# ============================================================
# host side
# ============================================================

_NC_CACHE = None
_NC_FLAGS = None


def _bf16(a):
    import ml_dtypes
    return np.ascontiguousarray(a.astype(ml_dtypes.bfloat16))


def _pack_qk(w):      # [D, DIM, 3072] -> [D, 16, P, 8, P]
    v = w[:, :, :2048].reshape(DEPTH, 8, P, 16, P).transpose(0, 3, 2, 1, 4)
    return _bf16(v)


def _pack_v(w):       # -> [D, 2, 8, P, 512]
    v = w[:, :, 2048:].reshape(DEPTH, 8, P, 2, 512).transpose(0, 3, 1, 2, 4)
    return _bf16(v)


def _pack_kxm(w):     # [D, K, M] -> [D, M//P, P, K//P, P]
    D, K, M = w.shape
    v = w.reshape(D, K // P, P, M // P, P).transpose(0, 3, 2, 1, 4)
    return _bf16(v)


def _pack_w2(w):      # [D, 4096, 1024] -> [D, 8, 2, P, 16, P]
    v = w.reshape(DEPTH, 2, 16, P, 8, P).transpose(0, 4, 1, 3, 2, 5)
    return _bf16(v)


def _host_band():
    tt = np.arange(KSIZE, dtype=np.float64)
    kern = np.exp(-0.5 * ((tt - (KSIZE - 1) / 2.0) / SIGMA) ** 2)
    kern = (kern / kern.sum()).astype(np.float32)
    pad_l = (KSIZE - 1) // 2  # 9
    nt = T // P
    bandc = np.zeros((nt, 3, P, P), dtype=np.float32)
    for ct in range(nt):
        for s in range(3):
            kt = ct - 1 + s
            if not (0 <= kt < nt):
                continue
            rows = np.arange(kt * P, (kt + 1) * P)
            cols = np.arange(ct * P, (ct + 1) * P)
            d = rows[:, None] - cols[None, :] + pad_l
            m = (d >= 0) & (d < KSIZE)
            blk = np.zeros((P, P), np.float32)
            blk[m] = kern[d[m]]
            bandc[ct, s] = blk
    return bandc


def _host_etab(rel_tab):
    i = np.arange(SEQ)
    j = i[:, None]
    rel = np.clip(i[None, :] - j, -(MAXREL - 1), MAXREL - 1) + MAXREL - 1
    et = np.zeros((DEPTH, SEQ, SEQ), dtype=np.float32)
    for l in range(DEPTH):
        e = np.exp(rel_tab[l][rel])
        e[j > i[None, :]] = 0.0
        et[l] = e
    return _bf16(et.reshape(DEPTH, SEQT, P, SEQ))


def kernel(**inputs):
    global _NC_CACHE, _NC_FLAGS

    f32 = lambda a: np.ascontiguousarray(np.asarray(a, dtype=np.float32))
    z = lambda a: bool(np.any(np.asarray(a) != 0))
    one = lambda a: bool(np.all(np.asarray(a) == 1.0))
    flags = {
        "gb_a": (not one(inputs["ln_a_g"])) or z(inputs["ln_a_b"]),
        "gb_f": (not one(inputs["ln_f_g"])) or z(inputs["ln_f_b"]),
        "gb_o": (not one(inputs["ln_o_g"])) or z(inputs["ln_o_b"]),
        "gb_p1": (not one(inputs["ln_p1_g"])) or z(inputs["ln_p1_b"]),
        "gb_p2": (not one(inputs["ln_p2_g"])) or z(inputs["ln_p2_b"]),
        "bo_nz": z(inputs["bo"]),
        "b2_nz": z(inputs["b2"]),
    }
    if _NC_CACHE is None or _NC_FLAGS != flags:
        _NC_CACHE = build_nc(flags)
        _NC_FLAGS = dict(flags)
    nc = _NC_CACHE

    def vx(a):       # [width] -> [P, width//P]  (partition-major)
        a = f32(a)
        return np.ascontiguousarray(a.reshape(-1, P).T)

    def vxl(a):      # [L, width] -> [P, L, width//P]
        a = f32(a)
        L = a.shape[0]
        return np.ascontiguousarray(a.reshape(L, -1, P).transpose(2, 0, 1))

    shared = {
        "band": _host_band(),
        "etab": _host_etab(f32(inputs["rel_tab"])),
        "wpe": f32(inputs["W_pe"]),
        "wqk_t": _pack_qk(f32(inputs["Wqkv"])),
        "wv_t": _pack_v(f32(inputs["Wqkv"])),
        "wot": _pack_kxm(f32(inputs["Wo"])),
        "w1t": _pack_kxm(f32(inputs["W1"])),
        "w2t": _pack_w2(f32(inputs["W2"])),
        "wproj": _bf16(f32(inputs["Wproj"])),
        "lnp1g": vx(inputs["ln_p1_g"]), "lnp1b": vx(inputs["ln_p1_b"]),
        "bpe": vx(inputs["b_pe"]),
        "lnp2g": vx(inputs["ln_p2_g"]), "lnp2b": vx(inputs["ln_p2_b"]),
        "lnag": vxl(inputs["ln_a_g"]), "lnab": vxl(inputs["ln_a_b"]),
        "lnfg": vxl(inputs["ln_f_g"]), "lnfb": vxl(inputs["ln_f_b"]),
        "bov": vxl(inputs["bo"]), "b1v": vxl(inputs["b1"]),
        "b2v": vxl(inputs["b2"]),
        "lnog": vx(inputs["ln_o_g"]), "lnob": vx(inputs["ln_o_b"]),
        "bprojv": f32(inputs["bproj"]),
    }
    xfull = f32(inputs["neuralInput"])
    in_maps = []
    for c in range(NCORES):
        m = dict(shared)
        m["xin"] = np.ascontiguousarray(xfull[c * BPC:(c + 1) * BPC])
        in_maps.append(m)

    import os
    trace = bool(os.environ.get("BIT_TRACE"))
    res = run_bass_kernel_spmd(nc, in_maps, list(range(NCORES)), trace=trace)
    if trace:
        globals()["LAST_RESULT"] = res
    outs = []
    for c in range(NCORES):
        o = res.results[c]["out"]              # [NCLS, TOK]
        o = o.reshape(NCLS, BPC, SEQ).transpose(1, 2, 0)
        outs.append(o)
    return np.concatenate(outs, axis=0).astype(np.float32)
